# revision 8
# baseline (speedup 1.0000x reference)
"""ATAE-LSTM Trainium2 kernel (8 NeuronCores, batch data-parallel).

Layout strategy (per core, local batch BL=32):
  - compute-heavy tensors live in "transposed" feature-major layouts:
      X^T  [128(p), K, COLS]   X^T[p,k,t*32+b]    = x[b,t,k*128+p]        (bf16)
      H^T  [128(p), K, COLS]   H^T[p,k,b*128+t]   = h_t[b,k*128+p]        (bf16)
      gates^T psum [128, JC, 32]  = gates[b, jc*128+p]                    (f32)
  - weights are passed pre-rearranged as SBUF images [128, k, jc, 128]
    so every matmul keeps the contraction dim on partitions with the weight
    as the full-128x128 stationary operand.
  - the LSTM mask-blend is dropped entirely: attention alphas are 0 past the
    sequence end, and h_last is recovered with a one-hot matmul at t=len-1.
  - the embedding gather uses dma_gather(transpose=True) which lands rows
    directly in feature-major layout.  int16 index limit is handled by
    gathering from two half-tables and mask-combining.
"""

import os
import sys

sys.path.insert(0, "/opt/trn_rl_repo")

import numpy as np
import ml_dtypes

BF16NP = ml_dtypes.bfloat16

import concourse.bass as bass
import concourse.tile as tile
from concourse import bacc, mybir
from concourse.bass import ds, ts

F32 = mybir.dt.float32
BF = mybir.dt.bfloat16
I16 = mybir.dt.int16
AF = mybir.ActivationFunctionType

B, T, D, NCLS = 256, 128, 768, 3
VOCAB, TVOCAB = 50000, 5000
NCORES = 8
BL = B // NCORES          # 32 local batch
K = D // 128              # 6 contraction chunks
JC = 4 * D // 128         # 24 gate output chunks
VSPLIT = 32768            # int16-safe table split


def _cols(t_):
    return BL * t_


def _nct(t_):
    cols = _cols(t_)
    for nct in (8, 4, 2, 1):
        cw = cols // nct
        if cw >= 128 and cols % nct == 0:
            return nct
    return 1


def build(t_steps=T, taps=()):
    """Build the SPMD single-core program (same program on all 8 cores)."""
    TT = t_steps
    COLS = _cols(TT)
    NCT = _nct(TT)
    CW = COLS // NCT
    TPC = TT // NCT  # timesteps per column tile

    nc = bacc.Bacc("TRN2", target_bir_lowering=False, debug=False)

    dt = nc.dram_tensor
    emb = dt("emb", [VOCAB, D], BF, kind="ExternalInput")
    temb = dt("temb", [TVOCAB, D], BF, kind="ExternalInput")
    whh_d = dt("whh", [128, K * JC * 128], BF, kind="ExternalInput")
    wihx_d = dt("wihx", [128, K * JC * 128], BF, kind="ExternalInput")
    wiht_d = dt("wiht", [128, K * JC * 128], BF, kind="ExternalInput")
    wh_d = dt("wh", [128, K * K * 128], BF, kind="ExternalInput")
    wp_d = dt("wp", [128, K * K * 128], BF, kind="ExternalInput")
    wx_d = dt("wx", [128, K * K * 128], BF, kind="ExternalInput")
    wlin_d = dt("wlin", [128, K * NCLS], BF, kind="ExternalInput")
    wvec_d = dt("wvec", [128, K], BF, kind="ExternalInput")
    blstm_d = dt("blstm", [128, JC], F32, kind="ExternalInput")
    blin_d = dt("blin", [NCLS, 1], F32, kind="ExternalInput")
    ident_d = dt("ident", [128, 128], F32, kind="ExternalInput")
    idxlo_d = dt("idxlo", [128, COLS // 16], I16, kind="ExternalInput")
    idxhi_d = dt("idxhi", [128, COLS // 16], I16, kind="ExternalInput")
    m0_d = dt("m0", [128, COLS], BF, kind="ExternalInput")
    tidx_d = dt("tidx", [128, 8], I16, kind="ExternalInput")
    mbias_d = dt("mbias", [BL, TT], F32, kind="ExternalInput")
    onehot_d = dt("onehot", [128, BL], BF, kind="ExternalInput")
    out_d = dt("out", [NCLS, BL], F32, kind="ExternalOutput")

    tap_d = {}
    for name, shape, dtp in (
        ("XT", [128, K * COLS], BF),
        ("HT", [128, K * COLS], BF),
        ("txT", [128, K * 128], BF),
        ("tgate", [128, JC * BL], F32),
        ("xg", [JC, 128, COLS], BF),
        ("scores", [BL, TT], F32),
        ("alpha", [BL, TT], F32),
        ("rh", [64, D], F32),
        ("hstar", [128, K * BL], BF),
        ("g1", [128, JC * BL], F32),
        ("sigif1", [128, 2 * K * BL], F32),
        ("c1", [128, K * BL], F32),
        ("psg1", [128, JC * BL], F32),
    ):
        if name in taps:
            tap_d[name] = dt("tap_" + name, shape, dtp, kind="ExternalOutput")

    with tile.TileContext(nc) as tc:
        with (
            tc.tile_pool(name="consts", bufs=1) as consts,
            tc.tile_pool(name="dram", bufs=1, space="DRAM") as dramp,
            tc.tile_pool(name="pHT", bufs=1) as pHT,
        ):
            # ---- long-lived small constants ----
            def cload(pool, dtsr, shape, dtype):
                t = pool.tile(shape, dtype, tag=dtsr.name)
                nc.sync.dma_start(t[:], dtsr[:])
                return t

            blstm_sb = cload(consts, blstm_d, [128, JC], F32)
            blin_sb = cload(consts, blin_d, [NCLS, 1], F32)
            ident_sb = cload(consts, ident_d, [128, 128], F32)
            wvec_sb = cload(consts, wvec_d, [128, K], BF)
            wlin_sb = cload(consts, wlin_d, [128, K * NCLS], BF).rearrange(
                "p (k c) -> p k c", c=NCLS)
            mbias_sb = cload(consts, mbias_d, [BL, TT], F32)
            onehot_sb = cload(consts, onehot_d, [128, BL], BF)

            HT = pHT.tile([128, K, COLS], BF)
            Vh = HT[:].rearrange("p k (b t) -> p k b t", b=BL, t=TT)
            xg_dram = dramp.tile([JC, 128, COLS], BF)
            sc_dram = dramp.tile([1, COLS], F32)

            # ================= phase 1: tgate + X gather + xpre ============
            with (
                tc.tile_pool(name="ph1", bufs=1) as ph1,
                tc.tile_pool(name="ps1", bufs=8, space="PSUM") as ps1,
            ):
                tgate = ph1.tile([128, JC, BL], F32, tag="tgate")
                tgrep = ph1.tile([128, JC, TPC * BL], BF, tag="tgrep")

                # --- 1a: target-embedding gather + tgate (frees wiht after) --
                with tc.tile_pool(name="pwiht", bufs=1) as pwiht:
                    wiht_sb = pwiht.tile([128, K, JC, 128], BF)
                    nc.sync.dma_start(wiht_sb[:], wiht_d[:].rearrange(
                        "p (k jc m) -> p k jc m", k=K, jc=JC))
                    tidx_sb = cload(pwiht, tidx_d, [128, 8], I16)

                    txT = pwiht.tile([128, K, 128], BF)
                    nc.gpsimd.dma_gather(
                        txT[:], temb[:], tidx_sb[:, :], num_idxs=128,
                        num_idxs_reg=128, elem_size=D, transpose=True)
                    if "txT" in tap_d:
                        nc.sync.dma_start(
                            tap_d["txT"][:], txT[:].rearrange("p k c -> p (k c)"))

                    for jc in range(JC):
                        pt = ps1.tile([128, BL], F32, tag="ps")
                        for k in range(K):
                            nc.tensor.matmul(
                                pt[:], wiht_sb[:, k, jc, :], txT[:, k, :BL],
                                start=(k == 0), stop=(k == K - 1))
                        nc.vector.tensor_scalar_add(
                            tgate[:, jc, :], pt[:], blstm_sb[:, jc : jc + 1])
                    if "tgate" in tap_d:
                        nc.sync.dma_start(
                            tap_d["tgate"][:], tgate[:].rearrange("p a b -> p (a b)"))
                    for r in range(TPC):
                        nc.vector.tensor_copy(
                            tgrep[:, :, r * BL : (r + 1) * BL], tgate[:])

                # --- 1b: X gather + combine + xpre ---
                with (
                    tc.tile_pool(name="pwihx", bufs=1) as pwihx,
                    tc.tile_pool(name="pgat", bufs=1) as pgat,
                    tc.tile_pool(name="pxgs", bufs=3) as pxgs,
                ):
                    wihx_sb = pwihx.tile([128, K, JC, 128], BF)
                    nc.sync.dma_start(wihx_sb[:], wihx_d[:].rearrange(
                        "p (k jc m) -> p k jc m", k=K, jc=JC))
                    idxlo_sb = cload(pwihx, idxlo_d, [128, COLS // 16], I16)
                    idxhi_sb = cload(pwihx, idxhi_d, [128, COLS // 16], I16)
                    m0_sb = cload(pwihx, m0_d, [128, COLS], BF)
                    XT = pwihx.tile([128, K, COLS], BF)

                    emb_lo = emb[0:VSPLIT, :]
                    emb_hi = emb[VSPLIT:VOCAB, :]
                    for c in range(NCT):
                        g0 = pgat.tile([128, K, CW], BF, tag="g0")
                        g1 = pgat.tile([128, K, CW], BF, tag="g1")
                        iw = CW // 16
                        nc.gpsimd.dma_gather(
                            g0[:], emb_lo, idxlo_sb[:, c * iw : (c + 1) * iw],
                            num_idxs=CW, num_idxs_reg=CW, elem_size=D,
                            transpose=True)
                        nc.gpsimd.dma_gather(
                            g1[:], emb_hi, idxhi_sb[:, c * iw : (c + 1) * iw],
                            num_idxs=CW, num_idxs_reg=CW, elem_size=D,
                            transpose=True)
                        # X^T[:,k,cs] = g1 + (g0-g1)*m0
                        for k in range(K):
                            cs = slice(c * CW, (c + 1) * CW)
                            tmp = pgat.tile([128, CW], BF, tag="cmb")
                            nc.vector.tensor_sub(tmp[:], g0[:, k, :], g1[:, k, :])
                            nc.vector.tensor_mul(tmp[:], tmp[:], m0_sb[:, cs])
                            nc.vector.tensor_add(XT[:, k, cs], tmp[:], g1[:, k, :])
                    if "XT" in tap_d:
                        nc.sync.dma_start(
                            tap_d["XT"][:], XT[:].rearrange("p k c -> p (k c)"))

                    # xpre: xg^T[jc,(t,b)] = sum_k Wihx[k,jc]^T @ X^T[k] (+tg)
                    for jc in range(JC):
                        pcs = [ps1.tile([128, CW], F32, tag="ps", name=f"pcs{c_}")
                               for c_ in range(NCT)]
                        for k in range(K):
                            for c in range(NCT):
                                nc.tensor.matmul(
                                    pcs[c][:], wihx_sb[:, k, jc, :],
                                    XT[:, k, c * CW : (c + 1) * CW],
                                    start=(k == 0), stop=(k == K - 1))
                        for c in range(NCT):
                            xs = pxgs.tile([128, CW], BF, tag="xgs")
                            nc.vector.tensor_add(
                                xs[:].rearrange("p (t b) -> p t b", b=BL),
                                pcs[c][:].rearrange("p (t b) -> p t b", b=BL),
                                tgrep[:, jc, :].rearrange(
                                    "p (t b) -> p t b", b=BL))
                            nc.sync.dma_start(
                                xg_dram[jc, :, c * CW : (c + 1) * CW], xs[:])

            if "xg" in tap_d:
                nc.sync.dma_start(tap_d["xg"][:], xg_dram[:])

            # ================= phase 2: recurrence =========================
            with (
                tc.tile_pool(name="pwhh", bufs=1) as pwhh,
                tc.tile_pool(name="pxgb", bufs=2) as pxgb,
                tc.tile_pool(name="pcell", bufs=2) as pcell,
                tc.tile_pool(name="ps_g", bufs=2, space="PSUM") as ps_g,
            ):
                whh_sb = pwhh.tile([128, K, JC, 128], BF)
                nc.sync.dma_start(whh_sb[:], whh_d[:].rearrange(
                    "p (k jc m) -> p k jc m", k=K, jc=JC))

                c_prev = pcell.tile([128, K, BL], F32, tag="c")
                nc.vector.memset(c_prev[:], 0.0)

                xgb = None
                for t in range(TT):
                    ct, tl = divmod(t, TPC)
                    if tl == 0:
                        xgb = pxgb.tile([128, JC, CW], BF, tag="xgb")
                        nc.sync.dma_start(
                            xgb[:],
                            xg_dram[:, :, ct * CW : (ct + 1) * CW].rearrange(
                                "jc p c -> p jc c"))
                    xg_t = xgb[:, :, tl * BL : (tl + 1) * BL]  # [128, JC, BL]

                    gates = pcell.tile([128, JC, BL], F32, tag="gates")
                    if t == 0:
                        nc.vector.tensor_copy(gates[:], xg_t)
                    else:
                        psg = ps_g.tile([128, JC, BL], F32, tag="psg")
                        # NOTE: start=True clears has_written at PSUM *bank*
                        # granularity, so each jc's accumulation group must be
                        # issued consecutively (jc-outer, k-inner).
                        for jc in range(JC):
                            for k in range(K):
                                nc.tensor.matmul(
                                    psg[:, jc, :], whh_sb[:, k, jc, :],
                                    Vh[:, k, :, t - 1],
                                    start=(k == 0), stop=(k == K - 1))
                        nc.vector.tensor_add(gates[:], psg[:], xg_t)
                        if t == 1 and "psg1" in tap_d:
                            ptap = pcell.tile([128, JC * BL], F32, tag="ptap")
                            nc.vector.tensor_copy(ptap[:], psg[:])
                            nc.sync.dma_start(tap_d["psg1"][:], ptap[:])

                    gf = gates[:].rearrange("p jc b -> p (jc b)")
                    NIF = K * BL  # 192 elements per gate
                    sigif = pcell.tile([128, 2 * NIF], F32, tag="sigif")
                    nc.scalar.activation(sigif[:], gf[:, 0 : 2 * NIF], AF.Sigmoid)
                    tang = pcell.tile([128, NIF], F32, tag="tang")
                    nc.scalar.activation(tang[:], gf[:, 2 * NIF : 3 * NIF], AF.Tanh)
                    sigo = pcell.tile([128, NIF], F32, tag="sigo")
                    nc.scalar.activation(sigo[:], gf[:, 3 * NIF : 4 * NIF], AF.Sigmoid)
                    if t == 1 and "g1" in tap_d:
                        nc.sync.dma_start(tap_d["g1"][:], gf)
                        nc.sync.dma_start(tap_d["sigif1"][:], sigif[:])

                    t1 = pcell.tile([128, NIF], F32, tag="t1")
                    nc.vector.tensor_mul(t1[:], sigif[:, NIF : 2 * NIF],
                                         c_prev[:].rearrange("p k b -> p (k b)"))
                    t2 = pcell.tile([128, NIF], F32, tag="t2")
                    nc.vector.tensor_mul(t2[:], sigif[:, 0:NIF], tang[:])
                    c_new = pcell.tile([128, K, BL], F32, tag="c")
                    cf = c_new[:].rearrange("p k b -> p (k b)")
                    nc.vector.tensor_add(cf, t1[:], t2[:])
                    tanc = pcell.tile([128, NIF], F32, tag="tanc")
                    nc.scalar.activation(tanc[:], cf, AF.Tanh)
                    if t == 1 and "c1" in tap_d:
                        nc.sync.dma_start(tap_d["c1"][:], cf)
                    nc.vector.tensor_mul(
                        Vh[:, :, :, t],
                        sigo[:].rearrange("p (k b) -> p k b", b=BL),
                        tanc[:].rearrange("p (k b) -> p k b", b=BL))
                    c_prev = c_new

                if "HT" in tap_d:
                    nc.sync.dma_start(
                        tap_d["HT"][:], HT[:].rearrange("p k c -> p (k c)"))

            # ================= phase 3: attention + head ===================
            with (
                tc.tile_pool(name="pattw", bufs=1) as pattw,
                tc.tile_pool(name="pthw", bufs=1) as pthw,
                tc.tile_pool(name="prows", bufs=1) as prows,
                tc.tile_pool(name="pattn", bufs=1) as pattn,
                tc.tile_pool(name="ps3", bufs=4, space="PSUM") as ps3,
            ):
                wh_sb = pattw.tile([128, K, K, 128], BF, tag="wh")
                nc.sync.dma_start(wh_sb[:], wh_d[:].rearrange(
                    "p (k jc m) -> p k jc m", k=K, jc=K))
                wp_sb = pattw.tile([128, K, K, 128], BF, tag="wp")
                nc.sync.dma_start(wp_sb[:], wp_d[:].rearrange(
                    "p (k jc m) -> p k jc m", k=K, jc=K))
                wx_sb = pattw.tile([128, K, K, 128], BF, tag="wx")
                nc.sync.dma_start(wx_sb[:], wx_d[:].rearrange(
                    "p (k jc m) -> p k jc m", k=K, jc=K))

                # tanh(H Wh)^T  [128, K, COLS]
                thw = pthw.tile([128, K, COLS], BF)
                for jc in range(K):
                    for c in range(NCT):
                        ph = ps3.tile([128, CW], F32, tag="ps")
                        for k in range(K):
                            nc.tensor.matmul(
                                ph[:], wh_sb[:, k, jc, :],
                                HT[:, k, c * CW : (c + 1) * CW],
                                start=(k == 0), stop=(k == K - 1))
                        nc.scalar.activation(
                            thw[:, jc, c * CW : (c + 1) * CW], ph[:], AF.Tanh)

                # scores[(b,t)] = sum_{jc,p} thw[p,jc,col] * w[jc*128+p]
                scf = pattn.tile([1, COLS], F32, tag="scf")
                for c in range(NCT):
                    psc = ps3.tile([1, CW], F32, tag="ps")
                    for jc in range(K):
                        nc.tensor.matmul(
                            psc[:], wvec_sb[:, jc : jc + 1],
                            thw[:, jc, c * CW : (c + 1) * CW],
                            start=(jc == 0), stop=(jc == K - 1))
                    nc.scalar.copy(scf[:, c * CW : (c + 1) * CW], psc[:])

                # reshape [1,(b,t)] -> [b,t] via DRAM bounce, mask, softmax
                nc.sync.dma_start(sc_dram[:], scf[:])
                sc_bt = pattn.tile([BL, TT], F32, tag="scbt")
                nc.sync.dma_start(
                    sc_bt[:],
                    sc_dram[:].rearrange("o (b t) -> o b t", b=BL)[0])
                nc.vector.tensor_add(sc_bt[:], sc_bt[:], mbias_sb[:])
                if "scores" in tap_d:
                    nc.sync.dma_start(tap_d["scores"][:], sc_bt[:])
                mx = pattn.tile([BL, 1], F32, tag="mx")
                nc.vector.reduce_max(mx[:], sc_bt[:], axis=mybir.AxisListType.X)
                nc.vector.tensor_scalar_sub(sc_bt[:], sc_bt[:], mx[:])
                nc.scalar.activation(sc_bt[:], sc_bt[:], AF.Exp)
                sm = pattn.tile([BL, 1], F32, tag="sm")
                nc.vector.reduce_sum(sm[:], sc_bt[:], axis=mybir.AxisListType.X)
                nc.vector.reciprocal(sm[:], sm[:])
                nc.vector.tensor_scalar_mul(sc_bt[:], sc_bt[:], sm[:])
                if "alpha" in tap_d:
                    nc.sync.dma_start(tap_d["alpha"][:], sc_bt[:])

                # alpha^T via PE transpose
                pal = ps3.tile([TT, BL], F32, tag="ps")
                nc.tensor.transpose(pal[:], sc_bt[:], ident_sb[:BL, :BL])
                alphaT = pattn.tile([TT, BL], BF, tag="alphaT")
                nc.vector.tensor_copy(alphaT[:], pal[:])

                # block-diagonal [alpha | onehot] stationary: A [T, BL*64]
                A = pattn.tile([TT, BL * 64], BF, tag="A")
                nc.vector.memset(A[:], 0.0)
                nc.vector.tensor_copy(A[:, 0 : BL * 64 : 65], alphaT[:])
                nc.vector.tensor_copy(A[:, 32 : BL * 64 : 65], onehot_sb[:TT, :])

                # H rows per b (t on partitions) via DMA transpose
                hrows = prows.tile([TT, BL, D], BF)
                identb = None
                if TT % 128 != 0:
                    identb = pattn.tile([128, 128], BF, tag="identb")
                    nc.vector.tensor_copy(identb[:], ident_sb[:])
                for b in range(BL):
                    for k in range(K):
                        if TT % 128 == 0:
                            nc.sync.dma_start(
                                hrows[:, b, k * 128 : (k + 1) * 128],
                                HT[:, k, b * TT : (b + 1) * TT],
                                transpose=True)
                        else:
                            ptp = ps3.tile([TT, 128], BF, tag="psb",
                                           name="ptp")
                            nc.tensor.transpose(
                                ptp[:], HT[:, k, b * TT : (b + 1) * TT],
                                identb[:, :])
                            nc.vector.tensor_copy(
                                hrows[:, b, k * 128 : (k + 1) * 128], ptp[:])

                # [r ; h_last] rows = sum_b A_b^T @ Hrows_b
                pr1 = ps3.tile([64, 512], F32, tag="ps")
                pr2 = ps3.tile([64, D - 512], F32, tag="ps")
                for b in range(BL):
                    Ab = A[:, b * 64 : (b + 1) * 64]
                    nc.tensor.matmul(pr1[:], Ab, hrows[:, b, 0:512],
                                     start=(b == 0), stop=(b == BL - 1))
                    nc.tensor.matmul(pr2[:], Ab, hrows[:, b, 512:D],
                                     start=(b == 0), stop=(b == BL - 1))
                rh = pattn.tile([64, D], F32, tag="rh")
                nc.vector.tensor_copy(rh[:, 0:512], pr1[:])
                nc.vector.tensor_copy(rh[:, 512:D], pr2[:])
                if "rh" in tap_d:
                    nc.sync.dma_start(tap_d["rh"][:], rh[:])

                # rh^T [128, K, 64] via PE transposes
                rhT = pattn.tile([128, K, 64], BF, tag="rhT")
                for k in range(K):
                    prt = ps3.tile([128, 64], F32, tag="ps")
                    nc.tensor.transpose(
                        prt[:], rh[:, k * 128 : (k + 1) * 128],
                        ident_sb[:64, :64])
                    nc.vector.tensor_copy(rhT[:, k, :], prt[:])

                # h_star^T = tanh(Wp^T r^T + Wx^T hlast^T)
                hstarT = pattn.tile([128, K, BL], BF, tag="hstarT")
                for jc in range(K):
                    phs = ps3.tile([128, BL], F32, tag="ps")
                    for k in range(K):
                        nc.tensor.matmul(phs[:], wp_sb[:, k, jc, :],
                                         rhT[:, k, 0:BL],
                                         start=(k == 0), stop=False)
                    for k in range(K):
                        nc.tensor.matmul(phs[:], wx_sb[:, k, jc, :],
                                         rhT[:, k, BL:64],
                                         start=False, stop=(k == K - 1))
                    nc.scalar.activation(hstarT[:, jc, :], phs[:], AF.Tanh)
                if "hstar" in tap_d:
                    nc.sync.dma_start(
                        tap_d["hstar"][:],
                        hstarT[:].rearrange("p k b -> p (k b)"))

                # logits^T [3, BL]
                pl = ps3.tile([NCLS, BL], F32, tag="ps")
                for k in range(K):
                    nc.tensor.matmul(pl[:], wlin_sb[:, k, :], hstarT[:, k, :],
                                     start=(k == 0), stop=(k == K - 1))
                logit = pattn.tile([NCLS, BL], F32, tag="logit")
                nc.vector.tensor_scalar_add(logit[:], pl[:], blin_sb[:])
                nc.sync.dma_start(out_d[:], logit[:])

    nc.finalize()
    return nc


# ======================= host-side wrapper =============================

_CACHE = {}


def _img_kjc(w, jc):
    # [768, jc*128] -> [128, K*jc*128] SBUF image, [p, k, jc, m]
    k = w.shape[0] // 128
    return np.ascontiguousarray(
        w.reshape(k, 128, jc, 128).transpose(1, 0, 2, 3).reshape(128, -1))


def prep_inputs(sent, target, lens, emb, temb, W_ih, W_hh, b_lstm, Wh, Wv, w,
                Wp, Wx, W_lin, b_lin, t_steps=T):
    TT = t_steps
    COLS = _cols(TT)
    b16 = lambda x: np.asarray(x, np.float32).astype(BF16NP)

    shared = {
        "emb": b16(emb),
        "temb": b16(temb),
        "whh": _img_kjc(b16(W_hh), JC),
        "wihx": _img_kjc(b16(W_ih[:D]), JC),
        "wiht": _img_kjc(b16(W_ih[D:]), JC),
        "wh": _img_kjc(b16(Wh), K),
        "wp": _img_kjc(b16(Wp), K),
        "wx": _img_kjc(b16(Wx), K),
        "wlin": np.ascontiguousarray(
            b16(W_lin).reshape(K, 128, NCLS).transpose(1, 0, 2).reshape(128, -1)),
        "wvec": np.ascontiguousarray(b16(w[:D]).reshape(K, 128).T),
        "blstm": np.ascontiguousarray(
            np.asarray(b_lstm, np.float32).reshape(JC, 128).T),
        "blin": np.asarray(b_lin, np.float32).reshape(NCLS, 1),
        "ident": np.eye(128, dtype=np.float32),
    }

    sent = np.asarray(sent)
    target = np.asarray(target)
    lens = np.asarray(lens)

    def wrap16(flat):
        # [n] -> [128, n//16]: wrapped in 16 partitions, replicated into all
        # 8 GpSimd-core stripes (each Q7 core reads its own 16-partition band)
        return np.ascontiguousarray(np.tile(flat.reshape(-1, 16).T, (8, 1)))

    in_maps = []
    for c in range(NCORES):
        sl = slice(c * BL, (c + 1) * BL)
        s = sent[sl, :TT]
        flat = s.T.reshape(-1).astype(np.int64)  # col = t*BL + b
        lo = np.where(flat < VSPLIT, flat, 0).astype(np.int16)
        hi = np.where(flat >= VSPLIT, flat - VSPLIT, 0).astype(np.int16)
        m0 = np.broadcast_to(
            (flat < VSPLIT).astype(BF16NP)[None, :], (128, COLS)).copy()
        tflat = np.zeros(128, np.int64)
        tflat[:BL] = target[sl]
        ln = np.clip(lens[sl].astype(np.int64), 1, TT)
        mbias = np.where(np.arange(TT)[None, :] < ln[:, None], 0.0, -1e9
                         ).astype(np.float32)
        onehot = (np.arange(128)[:, None] == (ln - 1)[None, :]).astype(BF16NP)
        m = dict(shared)
        m.update({
            "idxlo": wrap16(lo), "idxhi": wrap16(hi), "m0": m0,
            "tidx": wrap16(tflat.astype(np.int16)),
            "mbias": mbias, "onehot": onehot,
        })
        in_maps.append(m)
    return in_maps


def _run(inputs, t_steps=T, taps=(), trace=False):
    from concourse import bass_utils

    if trace:
        _install_profile_shim()
    key = (t_steps, tuple(sorted(taps)))
    if key not in _CACHE:
        _CACHE[key] = build(t_steps=t_steps, taps=taps)
    nc = _CACHE[key]
    in_maps = prep_inputs(t_steps=t_steps, **inputs)
    res = bass_utils.run_bass_kernel_spmd(
        nc, in_maps, core_ids=list(range(NCORES)), trace=trace)
    logits = np.zeros((B, NCLS), np.float32)
    for c in range(NCORES):
        logits[c * BL : (c + 1) * BL] = res.results[c]["out"].T
    return logits, res


def kernel(**inputs):
    logits, _ = _run(inputs)
    return logits


def _install_profile_shim():
    import contextlib, ctypes, types
    import antenv

    if "antenv.axon_hooks" in sys.modules:
        return
    so = "/opt/axon/libaxon_pjrt.so"
    try:
        lib = ctypes.CDLL(so)
        lib.axon_start_nrt_profile.argtypes = [
            ctypes.POINTER(ctypes.c_int64), ctypes.c_size_t]
        lib.axon_start_nrt_profile.restype = ctypes.c_int64
        lib.axon_stop_nrt_profile.argtypes = [ctypes.c_char_p]
        lib.axon_stop_nrt_profile.restype = ctypes.c_int64
    except OSError:
        return

    @contextlib.contextmanager
    def _hook(output_dir, device_ids):
        import jax
        jax.devices()
        if device_ids:
            ids = (ctypes.c_int64 * len(device_ids))(*device_ids)
            rc = lib.axon_start_nrt_profile(ids, len(device_ids))
        else:
            rc = lib.axon_start_nrt_profile(None, 0)
        if rc != 0:
            raise RuntimeError(f"axon_start_nrt_profile rc={rc}")
        try:
            yield
        finally:
            n = lib.axon_stop_nrt_profile(str(output_dir).encode())
            print(f"ntff profile: {n} file(s) -> {output_dir}", file=sys.stderr)

    mod = types.ModuleType("antenv.axon_hooks")
    mod.get_axon_ntff_profile_hook = lambda: _hook
    mod.set_axon_ntff_profile_hook = lambda h: None
    sys.modules["antenv.axon_hooks"] = mod
    antenv.axon_hooks = mod


# revision 11
# speedup vs baseline: 1.0392x; 1.0392x over previous
"""ATAE-LSTM Trainium2 kernel (8 NeuronCores, batch data-parallel).

Layout strategy (per core, local batch BL=32):
  - compute-heavy tensors live in "transposed" feature-major layouts:
      X^T  [128(p), K, COLS]   X^T[p,k,t*32+b]    = x[b,t,k*128+p]        (bf16)
      H^T  [128(p), K, COLS]   H^T[p,k,b*128+t]   = h_t[b,k*128+p]        (bf16)
      gates^T psum [128, JC, 32]  = gates[b, jc*128+p]                    (f32)
  - weights are passed pre-rearranged as SBUF images [128, k, jc, 128]
    so every matmul keeps the contraction dim on partitions with the weight
    as the full-128x128 stationary operand.
  - the LSTM mask-blend is dropped entirely: attention alphas are 0 past the
    sequence end, and h_last is recovered with a one-hot matmul at t=len-1.
  - the embedding gather uses dma_gather(transpose=True) which lands rows
    directly in feature-major layout.  int16 index limit is handled by
    gathering from two half-tables and mask-combining.
"""

import os
import sys

sys.path.insert(0, "/opt/trn_rl_repo")

import numpy as np
import ml_dtypes

BF16NP = ml_dtypes.bfloat16

import concourse.bass as bass
import concourse.tile as tile
from concourse import bacc, mybir
from concourse.bass import ds, ts

F32 = mybir.dt.float32
BF = mybir.dt.bfloat16
I16 = mybir.dt.int16
AF = mybir.ActivationFunctionType

B, T, D, NCLS = 256, 128, 768, 3
VOCAB, TVOCAB = 50000, 5000
NCORES = 8
BL = B // NCORES          # 32 local batch
K = D // 128              # 6 contraction chunks
JC = 4 * D // 128         # 24 gate output chunks
VSPLIT = 32768            # int16-safe table split


def _cols(t_):
    return BL * t_


def _nct(t_):
    cols = _cols(t_)
    for nct in (8, 4, 2, 1):
        cw = cols // nct
        if cw >= 128 and cols % nct == 0:
            return nct
    return 1


def build(t_steps=T, taps=()):
    """Build the SPMD single-core program (same program on all 8 cores)."""
    TT = t_steps
    COLS = _cols(TT)
    NCT = _nct(TT)
    CW = COLS // NCT
    TPC = TT // NCT  # timesteps per column tile

    nc = bacc.Bacc("TRN2", target_bir_lowering=False, debug=False)

    dt = nc.dram_tensor
    emb = dt("emb", [VOCAB, D], BF, kind="ExternalInput")
    temb = dt("temb", [TVOCAB, D], BF, kind="ExternalInput")
    whh_d = dt("whh", [128, K * JC * 128], BF, kind="ExternalInput")
    wihx_d = dt("wihx", [128, K * JC * 128], BF, kind="ExternalInput")
    wiht_d = dt("wiht", [128, K * JC * 128], BF, kind="ExternalInput")
    wh_d = dt("wh", [128, K * K * 128], BF, kind="ExternalInput")
    wp_d = dt("wp", [128, K * K * 128], BF, kind="ExternalInput")
    wx_d = dt("wx", [128, K * K * 128], BF, kind="ExternalInput")
    wlin_d = dt("wlin", [128, K * NCLS], BF, kind="ExternalInput")
    wvec_d = dt("wvec", [128, K], BF, kind="ExternalInput")
    blstm_d = dt("blstm", [128, JC], F32, kind="ExternalInput")
    blin_d = dt("blin", [NCLS, 1], F32, kind="ExternalInput")
    ident_d = dt("ident", [128, 128], F32, kind="ExternalInput")
    idxlo_d = dt("idxlo", [128, COLS // 16], I16, kind="ExternalInput")
    idxhi_d = dt("idxhi", [128, COLS // 16], I16, kind="ExternalInput")
    m0_d = dt("m0", [128, COLS], BF, kind="ExternalInput")
    tidx_d = dt("tidx", [128, 8], I16, kind="ExternalInput")
    mbias_d = dt("mbias", [BL, TT], F32, kind="ExternalInput")
    onehot_d = dt("onehot", [128, BL], BF, kind="ExternalInput")
    out_d = dt("out", [NCLS, BL], F32, kind="ExternalOutput")

    tap_d = {}
    for name, shape, dtp in (
        ("XT", [128, K * COLS], BF),
        ("HT", [128, K * COLS], BF),
        ("txT", [128, K * 128], BF),
        ("tgate", [128, JC * BL], F32),
        ("xg", [JC, 128, COLS], BF),
        ("scores", [BL, TT], F32),
        ("alpha", [BL, TT], F32),
        ("rh", [64, D], F32),
        ("hstar", [128, K * BL], BF),
        ("g1", [128, JC * BL], F32),
        ("sigif1", [128, 2 * K * BL], F32),
        ("c1", [128, K * BL], F32),
        ("psg1", [128, JC * BL], F32),
    ):
        if name in taps:
            tap_d[name] = dt("tap_" + name, shape, dtp, kind="ExternalOutput")

    with tile.TileContext(nc) as tc:
        with (
            tc.tile_pool(name="consts", bufs=1) as consts,
            tc.tile_pool(name="dram", bufs=1, space="DRAM") as dramp,
            tc.tile_pool(name="pHT", bufs=1) as pHT,
        ):
            # ---- long-lived small constants ----
            def cload(pool, dtsr, shape, dtype):
                t = pool.tile(shape, dtype, tag=dtsr.name)
                nc.sync.dma_start(t[:], dtsr[:])
                return t

            blstm_sb = cload(consts, blstm_d, [128, JC], F32)
            blin_sb = cload(consts, blin_d, [NCLS, 1], F32)
            ident_sb = cload(consts, ident_d, [128, 128], F32)
            wvec_sb = cload(consts, wvec_d, [128, K], BF)
            wlin_sb = cload(consts, wlin_d, [128, K * NCLS], BF).rearrange(
                "p (k c) -> p k c", c=NCLS)
            mbias_sb = cload(consts, mbias_d, [BL, TT], F32)
            onehot_sb = cload(consts, onehot_d, [128, BL], BF)

            HT = pHT.tile([128, K, COLS], BF)
            Vh = HT[:].rearrange("p k (b t) -> p k b t", b=BL, t=TT)
            xg_dram = dramp.tile([JC, 128, COLS], BF)
            sc_dram = dramp.tile([1, COLS], F32)

            # ================= phase 1: tgate + X gather + xpre ============
            with (
                tc.tile_pool(name="ph1", bufs=1) as ph1,
                tc.tile_pool(name="ps1", bufs=8, space="PSUM") as ps1,
            ):
                tgate = ph1.tile([128, JC, BL], F32, tag="tgate")
                tgrep = ph1.tile([128, JC, TPC * BL], BF, tag="tgrep")

                # --- 1a: target-embedding gather + tgate (frees wiht after) --
                with tc.tile_pool(name="pwiht", bufs=1) as pwiht:
                    wiht_sb = pwiht.tile([128, K, JC, 128], BF)
                    nc.sync.dma_start(wiht_sb[:], wiht_d[:].rearrange(
                        "p (k jc m) -> p k jc m", k=K, jc=JC))
                    tidx_sb = cload(pwiht, tidx_d, [128, 8], I16)

                    txT = pwiht.tile([128, K, 128], BF)
                    nc.gpsimd.dma_gather(
                        txT[:], temb[:], tidx_sb[:, :], num_idxs=128,
                        num_idxs_reg=128, elem_size=D, transpose=True)
                    if "txT" in tap_d:
                        nc.sync.dma_start(
                            tap_d["txT"][:], txT[:].rearrange("p k c -> p (k c)"))

                    for jc in range(JC):
                        pt = ps1.tile([128, BL], F32, tag="ps")
                        for k in range(K):
                            nc.tensor.matmul(
                                pt[:], wiht_sb[:, k, jc, :], txT[:, k, :BL],
                                start=(k == 0), stop=(k == K - 1))
                        nc.vector.tensor_scalar_add(
                            tgate[:, jc, :], pt[:], blstm_sb[:, jc : jc + 1])
                    if "tgate" in tap_d:
                        nc.sync.dma_start(
                            tap_d["tgate"][:], tgate[:].rearrange("p a b -> p (a b)"))
                    for r in range(TPC):
                        nc.vector.tensor_copy(
                            tgrep[:, :, r * BL : (r + 1) * BL], tgate[:])

                # --- 1b: X gather + combine + xpre ---
                with (
                    tc.tile_pool(name="pwihx", bufs=1) as pwihx,
                    tc.tile_pool(name="pgat", bufs=1) as pgat,
                    tc.tile_pool(name="pxgs", bufs=3) as pxgs,
                ):
                    wihx_sb = pwihx.tile([128, K, JC, 128], BF)
                    nc.sync.dma_start(wihx_sb[:], wihx_d[:].rearrange(
                        "p (k jc m) -> p k jc m", k=K, jc=JC))
                    idxlo_sb = cload(pwihx, idxlo_d, [128, COLS // 16], I16)
                    idxhi_sb = cload(pwihx, idxhi_d, [128, COLS // 16], I16)
                    m0_sb = cload(pwihx, m0_d, [128, COLS], BF)
                    XT = pwihx.tile([128, K, COLS], BF)

                    emb_lo = emb[0:VSPLIT, :]
                    emb_hi = emb[VSPLIT:VOCAB, :]
                    for c in range(NCT):
                        g0 = pgat.tile([128, K, CW], BF, tag="g0")
                        g1 = pgat.tile([128, K, CW], BF, tag="g1")
                        iw = CW // 16
                        nc.gpsimd.dma_gather(
                            g0[:], emb_lo, idxlo_sb[:, c * iw : (c + 1) * iw],
                            num_idxs=CW, num_idxs_reg=CW, elem_size=D,
                            transpose=True)
                        nc.gpsimd.dma_gather(
                            g1[:], emb_hi, idxhi_sb[:, c * iw : (c + 1) * iw],
                            num_idxs=CW, num_idxs_reg=CW, elem_size=D,
                            transpose=True)
                        # X^T[:,k,cs] = g1 + (g0-g1)*m0
                        for k in range(K):
                            cs = slice(c * CW, (c + 1) * CW)
                            tmp = pgat.tile([128, CW], BF, tag="cmb")
                            nc.vector.tensor_sub(tmp[:], g0[:, k, :], g1[:, k, :])
                            nc.vector.tensor_mul(tmp[:], tmp[:], m0_sb[:, cs])
                            nc.vector.tensor_add(XT[:, k, cs], tmp[:], g1[:, k, :])
                    if "XT" in tap_d:
                        nc.sync.dma_start(
                            tap_d["XT"][:], XT[:].rearrange("p k c -> p (k c)"))

                    # xpre: xg^T[jc,(t,b)] = sum_k Wihx[k,jc]^T @ X^T[k] (+tg)
                    # split into column halves so the second half's gathers
                    # overlap the first half's matmuls
                    HC = max(1, NCT // 2)
                    for half in range(NCT // HC):
                        crange = range(half * HC, (half + 1) * HC)
                        for jc in range(JC):
                            pcs = [ps1.tile([128, CW], F32, tag="ps",
                                            name=f"pcs{c_}") for c_ in crange]
                            for k in range(K):
                                for i, c in enumerate(crange):
                                    nc.tensor.matmul(
                                        pcs[i][:], wihx_sb[:, k, jc, :],
                                        XT[:, k, c * CW : (c + 1) * CW],
                                        start=(k == 0), stop=(k == K - 1))
                            for i, c in enumerate(crange):
                                xs = pxgs.tile([128, CW], BF, tag="xgs")
                                nc.vector.tensor_add(
                                    xs[:].rearrange("p (t b) -> p t b", b=BL),
                                    pcs[i][:].rearrange("p (t b) -> p t b", b=BL),
                                    tgrep[:, jc, :].rearrange(
                                        "p (t b) -> p t b", b=BL))
                                nc.sync.dma_start(
                                    xg_dram[jc, :, c * CW : (c + 1) * CW], xs[:])

            if "xg" in tap_d:
                nc.sync.dma_start(tap_d["xg"][:], xg_dram[:])

            # ================= phase 2: recurrence =========================
            with (
                tc.tile_pool(name="pwhh", bufs=1) as pwhh,
                tc.tile_pool(name="pxgb", bufs=2) as pxgb,
                tc.tile_pool(name="pcell", bufs=2) as pcell,
                tc.tile_pool(name="ps_g", bufs=2, space="PSUM") as ps_g,
            ):
                whh_sb = pwhh.tile([128, K, JC, 128], BF)
                nc.sync.dma_start(whh_sb[:], whh_d[:].rearrange(
                    "p (k jc m) -> p k jc m", k=K, jc=JC))

                c_prev = pcell.tile([128, K, BL], F32, tag="c")
                nc.vector.memset(c_prev[:], 0.0)

                # channel-chunk groups: group g covers h/c chunks
                # cc in [g*CCW, (g+1)*CCW); gate column chunks jc = gate*K+cc.
                # Elementwise for group g runs while the PE computes group
                # g+1's matmuls; next step's k-chunk matmuls start as soon as
                # h chunk k lands.
                NG = 3 if K % 3 == 0 else 1
                CCW = K // NG
                xgb = None
                for t in range(TT):
                    ct, tl = divmod(t, TPC)
                    if tl == 0:
                        xgb = pxgb.tile([128, JC, CW], BF, tag="xgb")
                        nc.sync.dma_start(
                            xgb[:],
                            xg_dram[:, :, ct * CW : (ct + 1) * CW].rearrange(
                                "jc p c -> p jc c"))
                    # [128, 4(gate), K(cc), BL]
                    xg_t = xgb[:, :, tl * BL : (tl + 1) * BL].rearrange(
                        "p (g cc) b -> p g cc b", g=4)

                    gates = pcell.tile([128, 4, K, BL], F32, tag="gates")
                    psg = None
                    if t == 0:
                        nc.vector.tensor_copy(gates[:], xg_t)
                    else:
                        psg = ps_g.tile([128, 4, K, BL], F32, tag="psg")
                        # NOTE: start=True clears has_written at PSUM *bank*
                        # granularity, so each jc\'s 6-matmul accumulation
                        # group must be issued consecutively.
                        for g in range(NG):
                            for gate in range(4):
                                for cc in range(g * CCW, (g + 1) * CCW):
                                    jc = gate * K + cc
                                    for k in range(K):
                                        nc.tensor.matmul(
                                            psg[:, gate, cc, :],
                                            whh_sb[:, k, jc, :],
                                            Vh[:, k, :, t - 1],
                                            start=(k == 0), stop=(k == K - 1))

                    c_new = pcell.tile([128, K, BL], F32, tag="c")
                    for g in range(NG):
                        sl = slice(g * CCW, (g + 1) * CCW)
                        if t > 0:
                            nc.vector.tensor_add(
                                gates[:, :, sl, :], psg[:, :, sl, :],
                                xg_t[:, :, sl, :])
                        sigif = pcell.tile([128, 2, CCW, BL], F32,
                                           tag="sigif", bufs=3)
                        nc.scalar.activation(sigif[:], gates[:, 0:2, sl, :],
                                             AF.Sigmoid)
                        tang = pcell.tile([128, CCW, BL], F32, tag="tang",
                                          bufs=3)
                        nc.scalar.activation(tang[:], gates[:, 2, sl, :],
                                             AF.Tanh)
                        sigo = pcell.tile([128, CCW, BL], F32, tag="sigo",
                                          bufs=3)
                        nc.scalar.activation(sigo[:], gates[:, 3, sl, :],
                                             AF.Sigmoid)
                        t1 = pcell.tile([128, CCW, BL], F32, tag="t1", bufs=3)
                        nc.vector.tensor_mul(t1[:], sigif[:, 1],
                                             c_prev[:, sl, :])
                        t2 = pcell.tile([128, CCW, BL], F32, tag="t2", bufs=3)
                        nc.vector.tensor_mul(t2[:], sigif[:, 0], tang[:])
                        nc.vector.tensor_add(c_new[:, sl, :], t1[:], t2[:])
                        tanc = pcell.tile([128, CCW, BL], F32, tag="tanc",
                                          bufs=3)
                        nc.scalar.activation(tanc[:], c_new[:, sl, :], AF.Tanh)
                        nc.vector.tensor_mul(Vh[:, sl, :, t], sigo[:], tanc[:])
                    c_prev = c_new

                if "HT" in tap_d:
                    nc.sync.dma_start(
                        tap_d["HT"][:], HT[:].rearrange("p k c -> p (k c)"))

            # ================= phase 3: attention + head ===================
            with (
                tc.tile_pool(name="pattw", bufs=1) as pattw,
                tc.tile_pool(name="pthw", bufs=1) as pthw,
                tc.tile_pool(name="prows", bufs=1) as prows,
                tc.tile_pool(name="pattn", bufs=1) as pattn,
                tc.tile_pool(name="ps3", bufs=4, space="PSUM") as ps3,
            ):
                wh_sb = pattw.tile([128, K, K, 128], BF, tag="wh")
                nc.sync.dma_start(wh_sb[:], wh_d[:].rearrange(
                    "p (k jc m) -> p k jc m", k=K, jc=K))
                wp_sb = pattw.tile([128, K, K, 128], BF, tag="wp")
                nc.sync.dma_start(wp_sb[:], wp_d[:].rearrange(
                    "p (k jc m) -> p k jc m", k=K, jc=K))
                wx_sb = pattw.tile([128, K, K, 128], BF, tag="wx")
                nc.sync.dma_start(wx_sb[:], wx_d[:].rearrange(
                    "p (k jc m) -> p k jc m", k=K, jc=K))

                # H rows per b (t on partitions) via DMA transpose,
                # split across both HWDGE queues; overlaps the HWh matmuls
                hrows = prows.tile([TT, BL, D], BF)
                identb = None
                if TT % 128 != 0:
                    identb = pattn.tile([128, 128], BF, tag="identb")
                    nc.vector.tensor_copy(identb[:], ident_sb[:])
                for b in range(BL):
                    for k in range(K):
                        if TT % 128 == 0:
                            eng = nc.sync if (b * K + k) % 2 == 0 else nc.scalar
                            eng.dma_start(
                                hrows[:, b, k * 128 : (k + 1) * 128],
                                HT[:, k, b * TT : (b + 1) * TT],
                                transpose=True)
                        else:
                            ptp = ps3.tile([TT, 128], BF, tag="psb",
                                           name="ptp")
                            nc.tensor.transpose(
                                ptp[:], HT[:, k, b * TT : (b + 1) * TT],
                                identb[:, :])
                            nc.vector.tensor_copy(
                                hrows[:, b, k * 128 : (k + 1) * 128], ptp[:])

                # tanh(H Wh)^T  [128, K, COLS]
                thw = pthw.tile([128, K, COLS], BF)
                for jc in range(K):
                    for c in range(NCT):
                        ph = ps3.tile([128, CW], F32, tag="ps")
                        for k in range(K):
                            nc.tensor.matmul(
                                ph[:], wh_sb[:, k, jc, :],
                                HT[:, k, c * CW : (c + 1) * CW],
                                start=(k == 0), stop=(k == K - 1))
                        nc.scalar.activation(
                            thw[:, jc, c * CW : (c + 1) * CW], ph[:], AF.Tanh)

                # scores[(b,t)] = sum_{jc,p} thw[p,jc,col] * w[jc*128+p]
                scf = pattn.tile([1, COLS], F32, tag="scf")
                for c in range(NCT):
                    psc = ps3.tile([1, CW], F32, tag="ps")
                    for jc in range(K):
                        nc.tensor.matmul(
                            psc[:], wvec_sb[:, jc : jc + 1],
                            thw[:, jc, c * CW : (c + 1) * CW],
                            start=(jc == 0), stop=(jc == K - 1))
                    nc.scalar.copy(scf[:, c * CW : (c + 1) * CW], psc[:])

                # reshape [1,(b,t)] -> [b,t] via DRAM bounce, mask, softmax
                nc.sync.dma_start(sc_dram[:], scf[:])
                sc_bt = pattn.tile([BL, TT], F32, tag="scbt")
                nc.sync.dma_start(
                    sc_bt[:],
                    sc_dram[:].rearrange("o (b t) -> o b t", b=BL)[0])
                nc.vector.tensor_add(sc_bt[:], sc_bt[:], mbias_sb[:])
                if "scores" in tap_d:
                    nc.sync.dma_start(tap_d["scores"][:], sc_bt[:])
                mx = pattn.tile([BL, 1], F32, tag="mx")
                nc.vector.reduce_max(mx[:], sc_bt[:], axis=mybir.AxisListType.X)
                nc.vector.tensor_scalar_sub(sc_bt[:], sc_bt[:], mx[:])
                nc.scalar.activation(sc_bt[:], sc_bt[:], AF.Exp)
                sm = pattn.tile([BL, 1], F32, tag="sm")
                nc.vector.reduce_sum(sm[:], sc_bt[:], axis=mybir.AxisListType.X)
                nc.vector.reciprocal(sm[:], sm[:])
                nc.vector.tensor_scalar_mul(sc_bt[:], sc_bt[:], sm[:])
                if "alpha" in tap_d:
                    nc.sync.dma_start(tap_d["alpha"][:], sc_bt[:])

                # alpha^T via PE transpose
                pal = ps3.tile([TT, BL], F32, tag="ps")
                nc.tensor.transpose(pal[:], sc_bt[:], ident_sb[:BL, :BL])
                alphaT = pattn.tile([TT, BL], BF, tag="alphaT")
                nc.vector.tensor_copy(alphaT[:], pal[:])

                # block-diagonal [alpha | onehot] stationary: A [T, BL*64]
                A = pattn.tile([TT, BL * 64], BF, tag="A")
                nc.vector.memset(A[:], 0.0)
                nc.vector.tensor_copy(A[:, 0 : BL * 64 : 65], alphaT[:])
                nc.vector.tensor_copy(A[:, 32 : BL * 64 : 65], onehot_sb[:TT, :])

                # [r ; h_last] rows = sum_b A_b^T @ Hrows_b
                pr1 = ps3.tile([64, 512], F32, tag="ps")
                pr2 = ps3.tile([64, D - 512], F32, tag="ps")
                for b in range(BL):
                    Ab = A[:, b * 64 : (b + 1) * 64]
                    nc.tensor.matmul(pr1[:], Ab, hrows[:, b, 0:512],
                                     start=(b == 0), stop=(b == BL - 1))
                    nc.tensor.matmul(pr2[:], Ab, hrows[:, b, 512:D],
                                     start=(b == 0), stop=(b == BL - 1))
                rh = pattn.tile([64, D], F32, tag="rh")
                nc.vector.tensor_copy(rh[:, 0:512], pr1[:])
                nc.vector.tensor_copy(rh[:, 512:D], pr2[:])
                if "rh" in tap_d:
                    nc.sync.dma_start(tap_d["rh"][:], rh[:])

                # rh^T [128, K, 64] via PE transposes
                rhT = pattn.tile([128, K, 64], BF, tag="rhT")
                for k in range(K):
                    prt = ps3.tile([128, 64], F32, tag="ps")
                    nc.tensor.transpose(
                        prt[:], rh[:, k * 128 : (k + 1) * 128],
                        ident_sb[:64, :64])
                    nc.vector.tensor_copy(rhT[:, k, :], prt[:])

                # h_star^T = tanh(Wp^T r^T + Wx^T hlast^T)
                hstarT = pattn.tile([128, K, BL], BF, tag="hstarT")
                for jc in range(K):
                    phs = ps3.tile([128, BL], F32, tag="ps")
                    for k in range(K):
                        nc.tensor.matmul(phs[:], wp_sb[:, k, jc, :],
                                         rhT[:, k, 0:BL],
                                         start=(k == 0), stop=False)
                    for k in range(K):
                        nc.tensor.matmul(phs[:], wx_sb[:, k, jc, :],
                                         rhT[:, k, BL:64],
                                         start=False, stop=(k == K - 1))
                    nc.scalar.activation(hstarT[:, jc, :], phs[:], AF.Tanh)
                if "hstar" in tap_d:
                    nc.sync.dma_start(
                        tap_d["hstar"][:],
                        hstarT[:].rearrange("p k b -> p (k b)"))

                # logits^T [3, BL]
                pl = ps3.tile([NCLS, BL], F32, tag="ps")
                for k in range(K):
                    nc.tensor.matmul(pl[:], wlin_sb[:, k, :], hstarT[:, k, :],
                                     start=(k == 0), stop=(k == K - 1))
                logit = pattn.tile([NCLS, BL], F32, tag="logit")
                nc.vector.tensor_scalar_add(logit[:], pl[:], blin_sb[:])
                nc.sync.dma_start(out_d[:], logit[:])

    nc.finalize()
    return nc


# ======================= host-side wrapper =============================

_CACHE = {}


def _img_kjc(w, jc):
    # [768, jc*128] -> [128, K*jc*128] SBUF image, [p, k, jc, m]
    k = w.shape[0] // 128
    return np.ascontiguousarray(
        w.reshape(k, 128, jc, 128).transpose(1, 0, 2, 3).reshape(128, -1))


def prep_inputs(sent, target, lens, emb, temb, W_ih, W_hh, b_lstm, Wh, Wv, w,
                Wp, Wx, W_lin, b_lin, t_steps=T):
    TT = t_steps
    COLS = _cols(TT)
    b16 = lambda x: np.asarray(x, np.float32).astype(BF16NP)

    shared = {
        "emb": b16(emb),
        "temb": b16(temb),
        "whh": _img_kjc(b16(W_hh), JC),
        "wihx": _img_kjc(b16(W_ih[:D]), JC),
        "wiht": _img_kjc(b16(W_ih[D:]), JC),
        "wh": _img_kjc(b16(Wh), K),
        "wp": _img_kjc(b16(Wp), K),
        "wx": _img_kjc(b16(Wx), K),
        "wlin": np.ascontiguousarray(
            b16(W_lin).reshape(K, 128, NCLS).transpose(1, 0, 2).reshape(128, -1)),
        "wvec": np.ascontiguousarray(b16(w[:D]).reshape(K, 128).T),
        "blstm": np.ascontiguousarray(
            np.asarray(b_lstm, np.float32).reshape(JC, 128).T),
        "blin": np.asarray(b_lin, np.float32).reshape(NCLS, 1),
        "ident": np.eye(128, dtype=np.float32),
    }

    sent = np.asarray(sent)
    target = np.asarray(target)
    lens = np.asarray(lens)

    def wrap16(flat):
        # [n] -> [128, n//16]: wrapped in 16 partitions, replicated into all
        # 8 GpSimd-core stripes (each Q7 core reads its own 16-partition band)
        return np.ascontiguousarray(np.tile(flat.reshape(-1, 16).T, (8, 1)))

    in_maps = []
    for c in range(NCORES):
        sl = slice(c * BL, (c + 1) * BL)
        s = sent[sl, :TT]
        flat = s.T.reshape(-1).astype(np.int64)  # col = t*BL + b
        lo = np.where(flat < VSPLIT, flat, 0).astype(np.int16)
        hi = np.where(flat >= VSPLIT, flat - VSPLIT, 0).astype(np.int16)
        m0 = np.broadcast_to(
            (flat < VSPLIT).astype(BF16NP)[None, :], (128, COLS)).copy()
        tflat = np.zeros(128, np.int64)
        tflat[:BL] = target[sl]
        ln = np.clip(lens[sl].astype(np.int64), 1, TT)
        mbias = np.where(np.arange(TT)[None, :] < ln[:, None], 0.0, -1e9
                         ).astype(np.float32)
        onehot = (np.arange(128)[:, None] == (ln - 1)[None, :]).astype(BF16NP)
        m = dict(shared)
        m.update({
            "idxlo": wrap16(lo), "idxhi": wrap16(hi), "m0": m0,
            "tidx": wrap16(tflat.astype(np.int16)),
            "mbias": mbias, "onehot": onehot,
        })
        in_maps.append(m)
    return in_maps


def _run(inputs, t_steps=T, taps=(), trace=False):
    from concourse import bass_utils

    if trace:
        _install_profile_shim()
    key = (t_steps, tuple(sorted(taps)))
    if key not in _CACHE:
        _CACHE[key] = build(t_steps=t_steps, taps=taps)
    nc = _CACHE[key]
    in_maps = prep_inputs(t_steps=t_steps, **inputs)
    res = bass_utils.run_bass_kernel_spmd(
        nc, in_maps, core_ids=list(range(NCORES)), trace=trace)
    logits = np.zeros((B, NCLS), np.float32)
    for c in range(NCORES):
        logits[c * BL : (c + 1) * BL] = res.results[c]["out"].T
    return logits, res


def kernel(**inputs):
    logits, _ = _run(inputs)
    return logits


def _install_profile_shim():
    import contextlib, ctypes, types
    import antenv

    if "antenv.axon_hooks" in sys.modules:
        return
    so = "/opt/axon/libaxon_pjrt.so"
    try:
        lib = ctypes.CDLL(so)
        lib.axon_start_nrt_profile.argtypes = [
            ctypes.POINTER(ctypes.c_int64), ctypes.c_size_t]
        lib.axon_start_nrt_profile.restype = ctypes.c_int64
        lib.axon_stop_nrt_profile.argtypes = [ctypes.c_char_p]
        lib.axon_stop_nrt_profile.restype = ctypes.c_int64
    except OSError:
        return

    @contextlib.contextmanager
    def _hook(output_dir, device_ids):
        import jax
        jax.devices()
        if device_ids:
            ids = (ctypes.c_int64 * len(device_ids))(*device_ids)
            rc = lib.axon_start_nrt_profile(ids, len(device_ids))
        else:
            rc = lib.axon_start_nrt_profile(None, 0)
        if rc != 0:
            raise RuntimeError(f"axon_start_nrt_profile rc={rc}")
        try:
            yield
        finally:
            n = lib.axon_stop_nrt_profile(str(output_dir).encode())
            print(f"ntff profile: {n} file(s) -> {output_dir}", file=sys.stderr)

    mod = types.ModuleType("antenv.axon_hooks")
    mod.get_axon_ntff_profile_hook = lambda: _hook
    mod.set_axon_ntff_profile_hook = lambda h: None
    sys.modules["antenv.axon_hooks"] = mod
    antenv.axon_hooks = mod


# revision 12
# speedup vs baseline: 1.1271x; 1.0846x over previous
"""ATAE-LSTM Trainium2 kernel (8 NeuronCores, batch data-parallel).

Layout strategy (per core, local batch BL=32):
  - compute-heavy tensors live in "transposed" feature-major layouts:
      X^T  [128(p), K, COLS]   X^T[p,k,t*32+b]    = x[b,t,k*128+p]        (bf16)
      H^T  [128(p), K, COLS]   H^T[p,k,b*128+t]   = h_t[b,k*128+p]        (bf16)
      gates^T psum [128, JC, 32]  = gates[b, jc*128+p]                    (f32)
  - weights are passed pre-rearranged as SBUF images [128, k, jc, 128]
    so every matmul keeps the contraction dim on partitions with the weight
    as the full-128x128 stationary operand.
  - the LSTM mask-blend is dropped entirely: attention alphas are 0 past the
    sequence end, and h_last is recovered with a one-hot matmul at t=len-1.
  - the embedding gather uses dma_gather(transpose=True) which lands rows
    directly in feature-major layout.  int16 index limit is handled by
    gathering from two half-tables and mask-combining.
"""

import os
import sys

sys.path.insert(0, "/opt/trn_rl_repo")

import numpy as np
import ml_dtypes

BF16NP = ml_dtypes.bfloat16

import concourse.bass as bass
import concourse.tile as tile
from concourse import bacc, mybir
from concourse.bass import ds, ts

F32 = mybir.dt.float32
BF = mybir.dt.bfloat16
I16 = mybir.dt.int16
AF = mybir.ActivationFunctionType

B, T, D, NCLS = 256, 128, 768, 3
VOCAB, TVOCAB = 50000, 5000
NCORES = 8
BL = B // NCORES          # 32 local batch
K = D // 128              # 6 contraction chunks
JC = 4 * D // 128         # 24 gate output chunks
VSPLIT = 32768            # int16-safe table split


def _cols(t_):
    return BL * t_


def _nct(t_):
    cols = _cols(t_)
    for nct in (8, 4, 2, 1):
        cw = cols // nct
        if cw >= 128 and cols % nct == 0:
            return nct
    return 1


def build(t_steps=T, taps=()):
    """Build the SPMD single-core program (same program on all 8 cores)."""
    TT = t_steps
    COLS = _cols(TT)
    NCT = _nct(TT)
    CW = COLS // NCT
    TPC = TT // NCT  # timesteps per column tile

    nc = bacc.Bacc("TRN2", target_bir_lowering=False, debug=False)

    dt = nc.dram_tensor
    emb = dt("emb", [VOCAB, D], BF, kind="ExternalInput")
    temb = dt("temb", [TVOCAB, D], BF, kind="ExternalInput")
    whh_d = dt("whh", [128, K * JC * 128], BF, kind="ExternalInput")
    wihx_d = dt("wihx", [128, K * JC * 128], BF, kind="ExternalInput")
    wiht_d = dt("wiht", [128, K * JC * 128], BF, kind="ExternalInput")
    wh_d = dt("wh", [128, K * K * 128], BF, kind="ExternalInput")
    wp_d = dt("wp", [128, K * K * 128], BF, kind="ExternalInput")
    wx_d = dt("wx", [128, K * K * 128], BF, kind="ExternalInput")
    wlin_d = dt("wlin", [128, K * NCLS], BF, kind="ExternalInput")
    wvec_d = dt("wvec", [128, K], BF, kind="ExternalInput")
    blstm_d = dt("blstm", [128, JC], F32, kind="ExternalInput")
    blin_d = dt("blin", [NCLS, 1], F32, kind="ExternalInput")
    ident_d = dt("ident", [128, 128], F32, kind="ExternalInput")
    idxlo_d = dt("idxlo", [128, COLS // 16], I16, kind="ExternalInput")
    idxhi_d = dt("idxhi", [128, COLS // 16], I16, kind="ExternalInput")
    m0_d = dt("m0", [128, COLS], BF, kind="ExternalInput")
    tidx_d = dt("tidx", [128, 8], I16, kind="ExternalInput")
    mbias_d = dt("mbias", [BL, TT], F32, kind="ExternalInput")
    onehot_d = dt("onehot", [128, BL], BF, kind="ExternalInput")
    out_d = dt("out", [NCLS, BL], F32, kind="ExternalOutput")

    tap_d = {}
    for name, shape, dtp in (
        ("XT", [128, K * COLS], BF),
        ("HT", [128, K * COLS], BF),
        ("txT", [128, K * 128], BF),
        ("tgate", [128, JC * BL], F32),
        ("xg", [JC, 128, COLS], BF),
        ("scores", [BL, TT], F32),
        ("alpha", [BL, TT], F32),
        ("rh", [64, D], F32),
        ("hstar", [128, K * BL], BF),
        ("g1", [128, JC * BL], F32),
        ("sigif1", [128, 2 * K * BL], F32),
        ("c1", [128, K * BL], F32),
        ("psg1", [128, JC * BL], F32),
    ):
        if name in taps:
            tap_d[name] = dt("tap_" + name, shape, dtp, kind="ExternalOutput")

    with tile.TileContext(nc) as tc:
        with (
            tc.tile_pool(name="consts", bufs=1) as consts,
            tc.tile_pool(name="dram", bufs=1, space="DRAM") as dramp,
            tc.tile_pool(name="pHT", bufs=1) as pHT,
        ):
            # ---- long-lived small constants ----
            def cload(pool, dtsr, shape, dtype):
                t = pool.tile(shape, dtype, tag=dtsr.name)
                nc.sync.dma_start(t[:], dtsr[:])
                return t

            blstm_sb = cload(consts, blstm_d, [128, JC], F32)
            blin_sb = cload(consts, blin_d, [NCLS, 1], F32)
            ident_sb = cload(consts, ident_d, [128, 128], F32)
            wvec_sb = cload(consts, wvec_d, [128, K], BF)
            wlin_sb = cload(consts, wlin_d, [128, K * NCLS], BF).rearrange(
                "p (k c) -> p k c", c=NCLS)
            mbias_sb = cload(consts, mbias_d, [BL, TT], F32)
            onehot_sb = cload(consts, onehot_d, [128, BL], BF)

            # H^T archive split into NG_H k-group tiles so next-step
            # matmuls depend only on the h chunks they actually read
            NG_H = 3 if K % 3 == 0 else 1
            CCW_H = K // NG_H
            HTg = [pHT.tile([128, CCW_H, COLS], BF, name=f"HT{g}", tag=f"HT{g}")
                   for g in range(NG_H)]
            Vhg = [h[:].rearrange("p k (b t) -> p k b t", b=BL, t=TT)
                   for h in HTg]

            def HTk(k):   # [128, COLS] view of chunk k
                return HTg[k // CCW_H][:, k % CCW_H, :]

            def Vhk(k):   # [128, BL, TT] view of chunk k
                return Vhg[k // CCW_H][:, k % CCW_H, :, :]
            xg_dram = dramp.tile([JC, 128, COLS], BF)
            sc_dram = dramp.tile([1, COLS], F32)

            # ================= phase 1: tgate + X gather + xpre ============
            with (
                tc.tile_pool(name="ph1", bufs=1) as ph1,
                tc.tile_pool(name="ps1", bufs=8, space="PSUM") as ps1,
            ):
                tgate = ph1.tile([128, JC, BL], F32, tag="tgate")
                tgrep = ph1.tile([128, JC, TPC * BL], BF, tag="tgrep")

                # --- 1a: target-embedding gather + tgate (frees wiht after) --
                with tc.tile_pool(name="pwiht", bufs=1) as pwiht:
                    wiht_sb = pwiht.tile([128, K, JC, 128], BF)
                    nc.sync.dma_start(wiht_sb[:], wiht_d[:].rearrange(
                        "p (k jc m) -> p k jc m", k=K, jc=JC))
                    tidx_sb = cload(pwiht, tidx_d, [128, 8], I16)

                    txT = pwiht.tile([128, K, 128], BF)
                    nc.gpsimd.dma_gather(
                        txT[:], temb[:], tidx_sb[:, :], num_idxs=128,
                        num_idxs_reg=128, elem_size=D, transpose=True)
                    if "txT" in tap_d:
                        nc.sync.dma_start(
                            tap_d["txT"][:], txT[:].rearrange("p k c -> p (k c)"))

                    for jc in range(JC):
                        pt = ps1.tile([128, BL], F32, tag="ps")
                        for k in range(K):
                            nc.tensor.matmul(
                                pt[:], wiht_sb[:, k, jc, :], txT[:, k, :BL],
                                start=(k == 0), stop=(k == K - 1))
                        nc.vector.tensor_scalar_add(
                            tgate[:, jc, :], pt[:], blstm_sb[:, jc : jc + 1])
                    if "tgate" in tap_d:
                        nc.sync.dma_start(
                            tap_d["tgate"][:], tgate[:].rearrange("p a b -> p (a b)"))
                    for r in range(TPC):
                        nc.vector.tensor_copy(
                            tgrep[:, :, r * BL : (r + 1) * BL], tgate[:])

                # --- 1b: X gather + combine + xpre ---
                with (
                    tc.tile_pool(name="pwihx", bufs=1) as pwihx,
                    tc.tile_pool(name="pgat", bufs=1) as pgat,
                    tc.tile_pool(name="pxgs", bufs=3) as pxgs,
                ):
                    wihx_sb = pwihx.tile([128, K, JC, 128], BF)
                    nc.sync.dma_start(wihx_sb[:], wihx_d[:].rearrange(
                        "p (k jc m) -> p k jc m", k=K, jc=JC))
                    idxlo_sb = cload(pwihx, idxlo_d, [128, COLS // 16], I16)
                    idxhi_sb = cload(pwihx, idxhi_d, [128, COLS // 16], I16)
                    m0_sb = cload(pwihx, m0_d, [128, COLS], BF)
                    XT = pwihx.tile([128, K, COLS], BF)

                    emb_lo = emb[0:VSPLIT, :]
                    emb_hi = emb[VSPLIT:VOCAB, :]
                    for c in range(NCT):
                        g0 = pgat.tile([128, K, CW], BF, tag="g0")
                        g1 = pgat.tile([128, K, CW], BF, tag="g1")
                        iw = CW // 16
                        nc.gpsimd.dma_gather(
                            g0[:], emb_lo, idxlo_sb[:, c * iw : (c + 1) * iw],
                            num_idxs=CW, num_idxs_reg=CW, elem_size=D,
                            transpose=True)
                        nc.gpsimd.dma_gather(
                            g1[:], emb_hi, idxhi_sb[:, c * iw : (c + 1) * iw],
                            num_idxs=CW, num_idxs_reg=CW, elem_size=D,
                            transpose=True)
                        # X^T[:,k,cs] = g1 + (g0-g1)*m0
                        for k in range(K):
                            cs = slice(c * CW, (c + 1) * CW)
                            tmp = pgat.tile([128, CW], BF, tag="cmb")
                            nc.vector.tensor_sub(tmp[:], g0[:, k, :], g1[:, k, :])
                            nc.vector.tensor_mul(tmp[:], tmp[:], m0_sb[:, cs])
                            nc.vector.tensor_add(XT[:, k, cs], tmp[:], g1[:, k, :])
                    if "XT" in tap_d:
                        nc.sync.dma_start(
                            tap_d["XT"][:], XT[:].rearrange("p k c -> p (k c)"))

                    # xpre: xg^T[jc,(t,b)] = sum_k Wihx[k,jc]^T @ X^T[k] (+tg)
                    # split into column halves so the second half's gathers
                    # overlap the first half's matmuls
                    HC = max(1, NCT // 2)
                    for half in range(NCT // HC):
                        crange = range(half * HC, (half + 1) * HC)
                        for jc in range(JC):
                            pcs = [ps1.tile([128, CW], F32, tag="ps",
                                            name=f"pcs{c_}") for c_ in crange]
                            for k in range(K):
                                for i, c in enumerate(crange):
                                    nc.tensor.matmul(
                                        pcs[i][:], wihx_sb[:, k, jc, :],
                                        XT[:, k, c * CW : (c + 1) * CW],
                                        start=(k == 0), stop=(k == K - 1))
                            for i, c in enumerate(crange):
                                xs = pxgs.tile([128, CW], BF, tag="xgs")
                                nc.vector.tensor_add(
                                    xs[:].rearrange("p (t b) -> p t b", b=BL),
                                    pcs[i][:].rearrange("p (t b) -> p t b", b=BL),
                                    tgrep[:, jc, :].rearrange(
                                        "p (t b) -> p t b", b=BL))
                                nc.sync.dma_start(
                                    xg_dram[jc, :, c * CW : (c + 1) * CW], xs[:])

            if "xg" in tap_d:
                nc.sync.dma_start(tap_d["xg"][:], xg_dram[:])

            # ================= phase 2: recurrence =========================
            with (
                tc.tile_pool(name="pwhh", bufs=1) as pwhh,
                tc.tile_pool(name="pxgb", bufs=2) as pxgb,
                tc.tile_pool(name="pcell", bufs=2) as pcell,
                tc.tile_pool(name="ps_g", bufs=2, space="PSUM") as ps_g,
            ):
                whh_sb = pwhh.tile([128, K, JC, 128], BF)
                nc.sync.dma_start(whh_sb[:], whh_d[:].rearrange(
                    "p (k jc m) -> p k jc m", k=K, jc=JC))

                # channel-chunk groups: group g covers h/c chunks
                # cc in [g*CCW,(g+1)*CCW); gate column chunks jc = gate*K+cc.
                # Elementwise for group g runs while the PE computes group
                # g+1's matmuls; next step's k-chunk matmuls start as soon
                # as h chunk k lands.
                NG = NG_H
                CCW = CCW_H

                c_prevs = []
                for g in range(NG):
                    cz = pcell.tile([128, K // NG, BL], F32, tag=f"c{g}",
                                    name="cz")
                    nc.vector.memset(cz[:], 0.0)
                    c_prevs.append(cz)

                xgb = None
                for t in range(TT):
                    ct, tl = divmod(t, TPC)
                    if tl == 0:
                        xgb = pxgb.tile([128, JC, CW], BF, tag="xgb")
                        nc.sync.dma_start(
                            xgb[:],
                            xg_dram[:, :, ct * CW : (ct + 1) * CW].rearrange(
                                "jc p c -> p jc c"))
                    # [128, 4(gate), K(cc), BL]
                    xg_t = xgb[:, :, tl * BL : (tl + 1) * BL].rearrange(
                        "p (g cc) b -> p g cc b", g=4)

                    psgs = [None] * NG
                    if t > 0:
                        # NOTE: start=True clears has_written at PSUM *bank*
                        # granularity, so each jc's 6-matmul accumulation
                        # group must be issued consecutively.
                        for g in range(NG):
                            psg = ps_g.tile([128, 4, CCW, BL], F32,
                                            tag=f"psg{g}", name=f"psg{g}")
                            psgs[g] = psg
                            for gate in range(4):
                                for ci, cc in enumerate(
                                        range(g * CCW, (g + 1) * CCW)):
                                    jc = gate * K + cc
                                    for k in range(K):
                                        nc.tensor.matmul(
                                            psg[:, gate, ci, :],
                                            whh_sb[:, k, jc, :],
                                            Vhk(k)[:, :, t - 1],
                                            start=(k == 0), stop=(k == K - 1))

                    c_news = []
                    for g in range(NG):
                        sl = slice(g * CCW, (g + 1) * CCW)
                        gates = pcell.tile([128, 4, CCW, BL], F32,
                                           tag=f"gates{g}", name=f"gates{g}")
                        if t == 0:
                            nc.vector.tensor_copy(gates[:], xg_t[:, :, sl, :])
                        else:
                            nc.vector.tensor_add(gates[:], psgs[g][:],
                                                 xg_t[:, :, sl, :])
                        sigif = pcell.tile([128, 2, CCW, BL], F32,
                                           tag=f"sigif{g}", name="sigif")
                        nc.scalar.activation(sigif[:], gates[:, 0:2],
                                             AF.Sigmoid)
                        tang = pcell.tile([128, CCW, BL], F32, tag=f"tang{g}",
                                          name="tang")
                        nc.scalar.activation(tang[:], gates[:, 2], AF.Tanh)
                        sigo = pcell.tile([128, CCW, BL], F32, tag=f"sigo{g}",
                                          name="sigo")
                        nc.scalar.activation(sigo[:], gates[:, 3], AF.Sigmoid)
                        t1 = pcell.tile([128, CCW, BL], F32, tag=f"t1{g}",
                                        name="t1")
                        nc.vector.tensor_mul(t1[:], sigif[:, 1], c_prevs[g][:])
                        t2 = pcell.tile([128, CCW, BL], F32, tag=f"t2{g}",
                                        name="t2")
                        nc.vector.tensor_mul(t2[:], sigif[:, 0], tang[:])
                        c_new = pcell.tile([128, CCW, BL], F32, tag=f"c{g}",
                                           name="c_new")
                        nc.vector.tensor_add(c_new[:], t1[:], t2[:])
                        tanc = pcell.tile([128, CCW, BL], F32, tag=f"tanc{g}",
                                          name="tanc")
                        nc.scalar.activation(tanc[:], c_new[:], AF.Tanh)
                        nc.vector.tensor_mul(
                            Vhg[g][:, :, :, t], sigo[:], tanc[:])
                        c_news.append(c_new)
                    c_prevs = c_news

                if "HT" in tap_d:
                    tapv = tap_d["HT"][:].rearrange(
                        "p (k c) -> p k c", k=K)
                    for k in range(K):
                        nc.sync.dma_start(tapv[:, k, :], HTk(k))

            # ================= phase 3: attention + head ===================
            with (
                tc.tile_pool(name="pattw", bufs=1) as pattw,
                tc.tile_pool(name="pthw", bufs=1) as pthw,
                tc.tile_pool(name="prows", bufs=1) as prows,
                tc.tile_pool(name="pattn", bufs=1) as pattn,
                tc.tile_pool(name="ps3", bufs=4, space="PSUM") as ps3,
            ):
                wh_sb = pattw.tile([128, K, K, 128], BF, tag="wh")
                nc.sync.dma_start(wh_sb[:], wh_d[:].rearrange(
                    "p (k jc m) -> p k jc m", k=K, jc=K))
                wp_sb = pattw.tile([128, K, K, 128], BF, tag="wp")
                nc.sync.dma_start(wp_sb[:], wp_d[:].rearrange(
                    "p (k jc m) -> p k jc m", k=K, jc=K))
                wx_sb = pattw.tile([128, K, K, 128], BF, tag="wx")
                nc.sync.dma_start(wx_sb[:], wx_d[:].rearrange(
                    "p (k jc m) -> p k jc m", k=K, jc=K))

                # H rows per b (t on partitions) via DMA transpose,
                # split across both HWDGE queues; overlaps the HWh matmuls
                hrows = prows.tile([TT, BL, D], BF)
                identb = None
                if TT % 128 != 0:
                    identb = pattn.tile([128, 128], BF, tag="identb")
                    nc.vector.tensor_copy(identb[:], ident_sb[:])
                for b in range(BL):
                    for k in range(K):
                        if TT % 128 == 0:
                            nc.sync.dma_start(
                                hrows[:, b, k * 128 : (k + 1) * 128],
                                HTk(k)[:, b * TT : (b + 1) * TT],
                                transpose=True)
                        else:
                            ptp = ps3.tile([TT, 128], BF, tag="psb",
                                           name="ptp")
                            nc.tensor.transpose(
                                ptp[:], HTk(k)[:, b * TT : (b + 1) * TT],
                                identb[:, :])
                            nc.vector.tensor_copy(
                                hrows[:, b, k * 128 : (k + 1) * 128], ptp[:])

                # tanh(H Wh)^T  [128, K, COLS]
                thw = pthw.tile([128, K, COLS], BF)
                for jc in range(K):
                    for c in range(NCT):
                        ph = ps3.tile([128, CW], F32, tag="ps")
                        for k in range(K):
                            nc.tensor.matmul(
                                ph[:], wh_sb[:, k, jc, :],
                                HTk(k)[:, c * CW : (c + 1) * CW],
                                start=(k == 0), stop=(k == K - 1))
                        nc.scalar.activation(
                            thw[:, jc, c * CW : (c + 1) * CW], ph[:], AF.Tanh)

                # scores[(b,t)] = sum_{jc,p} thw[p,jc,col] * w[jc*128+p]
                scf = pattn.tile([1, COLS], F32, tag="scf")
                for c in range(NCT):
                    psc = ps3.tile([1, CW], F32, tag="ps")
                    for jc in range(K):
                        nc.tensor.matmul(
                            psc[:], wvec_sb[:, jc : jc + 1],
                            thw[:, jc, c * CW : (c + 1) * CW],
                            start=(jc == 0), stop=(jc == K - 1))
                    nc.scalar.copy(scf[:, c * CW : (c + 1) * CW], psc[:])

                # reshape [1,(b,t)] -> [b,t] via DRAM bounce, mask, softmax
                nc.sync.dma_start(sc_dram[:], scf[:])
                sc_bt = pattn.tile([BL, TT], F32, tag="scbt")
                nc.sync.dma_start(
                    sc_bt[:],
                    sc_dram[:].rearrange("o (b t) -> o b t", b=BL)[0])
                nc.vector.tensor_add(sc_bt[:], sc_bt[:], mbias_sb[:])
                if "scores" in tap_d:
                    nc.sync.dma_start(tap_d["scores"][:], sc_bt[:])
                mx = pattn.tile([BL, 1], F32, tag="mx")
                nc.vector.reduce_max(mx[:], sc_bt[:], axis=mybir.AxisListType.X)
                nc.vector.tensor_scalar_sub(sc_bt[:], sc_bt[:], mx[:])
                nc.scalar.activation(sc_bt[:], sc_bt[:], AF.Exp)
                sm = pattn.tile([BL, 1], F32, tag="sm")
                nc.vector.reduce_sum(sm[:], sc_bt[:], axis=mybir.AxisListType.X)
                nc.vector.reciprocal(sm[:], sm[:])
                nc.vector.tensor_scalar_mul(sc_bt[:], sc_bt[:], sm[:])
                if "alpha" in tap_d:
                    nc.sync.dma_start(tap_d["alpha"][:], sc_bt[:])

                # alpha^T via PE transpose
                pal = ps3.tile([TT, BL], F32, tag="ps")
                nc.tensor.transpose(pal[:], sc_bt[:], ident_sb[:BL, :BL])
                alphaT = pattn.tile([TT, BL], BF, tag="alphaT")
                nc.vector.tensor_copy(alphaT[:], pal[:])

                # block-diagonal [alpha | onehot] stationary: A [T, BL*64]
                A = pattn.tile([TT, BL * 64], BF, tag="A")
                nc.vector.memset(A[:], 0.0)
                nc.vector.tensor_copy(A[:, 0 : BL * 64 : 65], alphaT[:])
                nc.vector.tensor_copy(A[:, 32 : BL * 64 : 65], onehot_sb[:TT, :])

                # [r ; h_last] rows = sum_b A_b^T @ Hrows_b
                pr1 = ps3.tile([64, 512], F32, tag="ps")
                pr2 = ps3.tile([64, D - 512], F32, tag="ps")
                for b in range(BL):
                    Ab = A[:, b * 64 : (b + 1) * 64]
                    nc.tensor.matmul(pr1[:], Ab, hrows[:, b, 0:512],
                                     start=(b == 0), stop=(b == BL - 1))
                    nc.tensor.matmul(pr2[:], Ab, hrows[:, b, 512:D],
                                     start=(b == 0), stop=(b == BL - 1))
                rh = pattn.tile([64, D], F32, tag="rh")
                nc.vector.tensor_copy(rh[:, 0:512], pr1[:])
                nc.vector.tensor_copy(rh[:, 512:D], pr2[:])
                if "rh" in tap_d:
                    nc.sync.dma_start(tap_d["rh"][:], rh[:])

                # rh^T [128, K, 64] via PE transposes
                rhT = pattn.tile([128, K, 64], BF, tag="rhT")
                for k in range(K):
                    prt = ps3.tile([128, 64], F32, tag="ps")
                    nc.tensor.transpose(
                        prt[:], rh[:, k * 128 : (k + 1) * 128],
                        ident_sb[:64, :64])
                    nc.vector.tensor_copy(rhT[:, k, :], prt[:])

                # h_star^T = tanh(Wp^T r^T + Wx^T hlast^T)
                hstarT = pattn.tile([128, K, BL], BF, tag="hstarT")
                for jc in range(K):
                    phs = ps3.tile([128, BL], F32, tag="ps")
                    for k in range(K):
                        nc.tensor.matmul(phs[:], wp_sb[:, k, jc, :],
                                         rhT[:, k, 0:BL],
                                         start=(k == 0), stop=False)
                    for k in range(K):
                        nc.tensor.matmul(phs[:], wx_sb[:, k, jc, :],
                                         rhT[:, k, BL:64],
                                         start=False, stop=(k == K - 1))
                    nc.scalar.activation(hstarT[:, jc, :], phs[:], AF.Tanh)
                if "hstar" in tap_d:
                    nc.sync.dma_start(
                        tap_d["hstar"][:],
                        hstarT[:].rearrange("p k b -> p (k b)"))

                # logits^T [3, BL]
                pl = ps3.tile([NCLS, BL], F32, tag="ps")
                for k in range(K):
                    nc.tensor.matmul(pl[:], wlin_sb[:, k, :], hstarT[:, k, :],
                                     start=(k == 0), stop=(k == K - 1))
                logit = pattn.tile([NCLS, BL], F32, tag="logit")
                nc.vector.tensor_scalar_add(logit[:], pl[:], blin_sb[:])
                nc.sync.dma_start(out_d[:], logit[:])

    nc.finalize()
    return nc


# ======================= host-side wrapper =============================

_CACHE = {}


def _img_kjc(w, jc):
    # [768, jc*128] -> [128, K*jc*128] SBUF image, [p, k, jc, m]
    k = w.shape[0] // 128
    return np.ascontiguousarray(
        w.reshape(k, 128, jc, 128).transpose(1, 0, 2, 3).reshape(128, -1))


def prep_inputs(sent, target, lens, emb, temb, W_ih, W_hh, b_lstm, Wh, Wv, w,
                Wp, Wx, W_lin, b_lin, t_steps=T):
    TT = t_steps
    COLS = _cols(TT)
    b16 = lambda x: np.asarray(x, np.float32).astype(BF16NP)

    shared = {
        "emb": b16(emb),
        "temb": b16(temb),
        "whh": _img_kjc(b16(W_hh), JC),
        "wihx": _img_kjc(b16(W_ih[:D]), JC),
        "wiht": _img_kjc(b16(W_ih[D:]), JC),
        "wh": _img_kjc(b16(Wh), K),
        "wp": _img_kjc(b16(Wp), K),
        "wx": _img_kjc(b16(Wx), K),
        "wlin": np.ascontiguousarray(
            b16(W_lin).reshape(K, 128, NCLS).transpose(1, 0, 2).reshape(128, -1)),
        "wvec": np.ascontiguousarray(b16(w[:D]).reshape(K, 128).T),
        "blstm": np.ascontiguousarray(
            np.asarray(b_lstm, np.float32).reshape(JC, 128).T),
        "blin": np.asarray(b_lin, np.float32).reshape(NCLS, 1),
        "ident": np.eye(128, dtype=np.float32),
    }

    sent = np.asarray(sent)
    target = np.asarray(target)
    lens = np.asarray(lens)

    def wrap16(flat):
        # [n] -> [128, n//16]: wrapped in 16 partitions, replicated into all
        # 8 GpSimd-core stripes (each Q7 core reads its own 16-partition band)
        return np.ascontiguousarray(np.tile(flat.reshape(-1, 16).T, (8, 1)))

    in_maps = []
    for c in range(NCORES):
        sl = slice(c * BL, (c + 1) * BL)
        s = sent[sl, :TT]
        flat = s.T.reshape(-1).astype(np.int64)  # col = t*BL + b
        lo = np.where(flat < VSPLIT, flat, 0).astype(np.int16)
        hi = np.where(flat >= VSPLIT, flat - VSPLIT, 0).astype(np.int16)
        m0 = np.broadcast_to(
            (flat < VSPLIT).astype(BF16NP)[None, :], (128, COLS)).copy()
        tflat = np.zeros(128, np.int64)
        tflat[:BL] = target[sl]
        ln = np.clip(lens[sl].astype(np.int64), 1, TT)
        mbias = np.where(np.arange(TT)[None, :] < ln[:, None], 0.0, -1e9
                         ).astype(np.float32)
        onehot = (np.arange(128)[:, None] == (ln - 1)[None, :]).astype(BF16NP)
        m = dict(shared)
        m.update({
            "idxlo": wrap16(lo), "idxhi": wrap16(hi), "m0": m0,
            "tidx": wrap16(tflat.astype(np.int16)),
            "mbias": mbias, "onehot": onehot,
        })
        in_maps.append(m)
    return in_maps


def _run(inputs, t_steps=T, taps=(), trace=False):
    from concourse import bass_utils

    if trace:
        _install_profile_shim()
    key = (t_steps, tuple(sorted(taps)))
    if key not in _CACHE:
        _CACHE[key] = build(t_steps=t_steps, taps=taps)
    nc = _CACHE[key]
    in_maps = prep_inputs(t_steps=t_steps, **inputs)
    res = bass_utils.run_bass_kernel_spmd(
        nc, in_maps, core_ids=list(range(NCORES)), trace=trace)
    logits = np.zeros((B, NCLS), np.float32)
    for c in range(NCORES):
        logits[c * BL : (c + 1) * BL] = res.results[c]["out"].T
    return logits, res


def kernel(**inputs):
    logits, _ = _run(inputs)
    return logits


def _install_profile_shim():
    import contextlib, ctypes, types
    import antenv

    if "antenv.axon_hooks" in sys.modules:
        return
    so = "/opt/axon/libaxon_pjrt.so"
    try:
        lib = ctypes.CDLL(so)
        lib.axon_start_nrt_profile.argtypes = [
            ctypes.POINTER(ctypes.c_int64), ctypes.c_size_t]
        lib.axon_start_nrt_profile.restype = ctypes.c_int64
        lib.axon_stop_nrt_profile.argtypes = [ctypes.c_char_p]
        lib.axon_stop_nrt_profile.restype = ctypes.c_int64
    except OSError:
        return

    @contextlib.contextmanager
    def _hook(output_dir, device_ids):
        import jax
        jax.devices()
        if device_ids:
            ids = (ctypes.c_int64 * len(device_ids))(*device_ids)
            rc = lib.axon_start_nrt_profile(ids, len(device_ids))
        else:
            rc = lib.axon_start_nrt_profile(None, 0)
        if rc != 0:
            raise RuntimeError(f"axon_start_nrt_profile rc={rc}")
        try:
            yield
        finally:
            n = lib.axon_stop_nrt_profile(str(output_dir).encode())
            print(f"ntff profile: {n} file(s) -> {output_dir}", file=sys.stderr)

    mod = types.ModuleType("antenv.axon_hooks")
    mod.get_axon_ntff_profile_hook = lambda: _hook
    mod.set_axon_ntff_profile_hook = lambda h: None
    sys.modules["antenv.axon_hooks"] = mod
    antenv.axon_hooks = mod


# revision 13
# speedup vs baseline: 1.1438x; 1.0148x over previous
"""ATAE-LSTM Trainium2 kernel (8 NeuronCores, batch data-parallel).

Layout strategy (per core, local batch BL=32):
  - compute-heavy tensors live in "transposed" feature-major layouts:
      X^T  [128(p), K, COLS]   X^T[p,k,t*32+b]    = x[b,t,k*128+p]        (bf16)
      H^T  [128(p), K, COLS]   H^T[p,k,b*128+t]   = h_t[b,k*128+p]        (bf16)
      gates^T psum [128, JC, 32]  = gates[b, jc*128+p]                    (f32)
  - weights are passed pre-rearranged as SBUF images [128, k, jc, 128]
    so every matmul keeps the contraction dim on partitions with the weight
    as the full-128x128 stationary operand.
  - the LSTM mask-blend is dropped entirely: attention alphas are 0 past the
    sequence end, and h_last is recovered with a one-hot matmul at t=len-1.
  - the embedding gather uses dma_gather(transpose=True) which lands rows
    directly in feature-major layout.  int16 index limit is handled by
    gathering from two half-tables and mask-combining.
"""

import os
import sys

sys.path.insert(0, "/opt/trn_rl_repo")

import numpy as np
import ml_dtypes

BF16NP = ml_dtypes.bfloat16

import concourse.bass as bass
import concourse.tile as tile
from concourse import bacc, mybir
from concourse.bass import ds, ts

F32 = mybir.dt.float32
BF = mybir.dt.bfloat16
F8 = mybir.dt.float8e4
F8NP = ml_dtypes.float8_e4m3
I16 = mybir.dt.int16
AF = mybir.ActivationFunctionType

B, T, D, NCLS = 256, 128, 768, 3
VOCAB, TVOCAB = 50000, 5000
NCORES = 8
BL = B // NCORES          # 32 local batch
K = D // 128              # 6 contraction chunks
JC = 4 * D // 128         # 24 gate output chunks
VSPLIT = 32768            # int16-safe table split


def _cols(t_):
    return BL * t_


def _nct(t_):
    cols = _cols(t_)
    for nct in (8, 4, 2, 1):
        cw = cols // nct
        if cw >= 128 and cols % nct == 0:
            return nct
    return 1


def build(t_steps=T, taps=()):
    """Build the SPMD single-core program (same program on all 8 cores)."""
    TT = t_steps
    COLS = _cols(TT)
    NCT = _nct(TT)
    CW = COLS // NCT
    TPC = TT // NCT  # timesteps per column tile

    nc = bacc.Bacc("TRN2", target_bir_lowering=False, debug=False)

    dt = nc.dram_tensor
    emb = dt("emb", [VOCAB, D], BF, kind="ExternalInput")
    temb = dt("temb", [TVOCAB, D], BF, kind="ExternalInput")
    whh_d = dt("whh", [128, K * JC * 128], F8, kind="ExternalInput")
    wihx_d = dt("wihx", [128, K * JC * 128], BF, kind="ExternalInput")
    wiht_d = dt("wiht", [128, K * JC * 128], BF, kind="ExternalInput")
    wh_d = dt("wh", [128, K * K * 128], BF, kind="ExternalInput")
    wp_d = dt("wp", [128, K * K * 128], BF, kind="ExternalInput")
    wx_d = dt("wx", [128, K * K * 128], BF, kind="ExternalInput")
    wlin_d = dt("wlin", [128, K * NCLS], BF, kind="ExternalInput")
    wvec_d = dt("wvec", [128, K], BF, kind="ExternalInput")
    blstm_d = dt("blstm", [128, JC], F32, kind="ExternalInput")
    blin_d = dt("blin", [NCLS, 1], F32, kind="ExternalInput")
    ident_d = dt("ident", [128, 128], F32, kind="ExternalInput")
    idxlo_d = dt("idxlo", [128, COLS // 16], I16, kind="ExternalInput")
    idxhi_d = dt("idxhi", [128, COLS // 16], I16, kind="ExternalInput")
    m0_d = dt("m0", [128, COLS], BF, kind="ExternalInput")
    tidx_d = dt("tidx", [128, 8], I16, kind="ExternalInput")
    mbias_d = dt("mbias", [BL, TT], F32, kind="ExternalInput")
    onehot_d = dt("onehot", [128, BL], BF, kind="ExternalInput")
    out_d = dt("out", [NCLS, BL], F32, kind="ExternalOutput")

    tap_d = {}
    for name, shape, dtp in (
        ("XT", [128, K * COLS], BF),
        ("HT", [128, K * COLS], BF),
        ("txT", [128, K * 128], BF),
        ("tgate", [128, JC * BL], F32),
        ("xg", [JC, 128, COLS], BF),
        ("scores", [BL, TT], F32),
        ("alpha", [BL, TT], F32),
        ("rh", [64, D], F32),
        ("hstar", [128, K * BL], BF),
        ("g1", [128, JC * BL], F32),
        ("sigif1", [128, 2 * K * BL], F32),
        ("c1", [128, K * BL], F32),
        ("psg1", [128, JC * BL], F32),
    ):
        if name in taps:
            tap_d[name] = dt("tap_" + name, shape, dtp, kind="ExternalOutput")

    with tile.TileContext(nc) as tc:
        with (
            tc.tile_pool(name="consts", bufs=1) as consts,
            tc.tile_pool(name="dram", bufs=1, space="DRAM") as dramp,
            tc.tile_pool(name="pHT", bufs=1) as pHT,
        ):
            # ---- long-lived small constants ----
            def cload(pool, dtsr, shape, dtype):
                t = pool.tile(shape, dtype, tag=dtsr.name)
                nc.sync.dma_start(t[:], dtsr[:])
                return t

            blstm_sb = cload(consts, blstm_d, [128, JC], F32)
            blin_sb = cload(consts, blin_d, [NCLS, 1], F32)
            ident_sb = cload(consts, ident_d, [128, 128], F32)
            wvec_sb = cload(consts, wvec_d, [128, K], BF)
            wlin_sb = cload(consts, wlin_d, [128, K * NCLS], BF).rearrange(
                "p (k c) -> p k c", c=NCLS)
            mbias_sb = cload(consts, mbias_d, [BL, TT], F32)
            onehot_sb = cload(consts, onehot_d, [128, BL], BF)

            # H^T archive split into NG_H k-group tiles so next-step
            # matmuls depend only on the h chunks they actually read
            NG_H = 3 if K % 3 == 0 else 1
            CCW_H = K // NG_H
            HTg = [pHT.tile([128, CCW_H, COLS], BF, name=f"HT{g}", tag=f"HT{g}")
                   for g in range(NG_H)]
            Vhg = [h[:].rearrange("p k (b t) -> p k b t", b=BL, t=TT)
                   for h in HTg]

            def HTk(k):   # [128, COLS] view of chunk k
                return HTg[k // CCW_H][:, k % CCW_H, :]

            def Vhk(k):   # [128, BL, TT] view of chunk k
                return Vhg[k // CCW_H][:, k % CCW_H, :, :]
            xg_dram = dramp.tile([JC, 128, COLS], BF)
            sc_dram = dramp.tile([1, COLS], F32)

            # ================= phase 1: tgate + X gather + xpre ============
            with (
                tc.tile_pool(name="ph1", bufs=1) as ph1,
                tc.tile_pool(name="ps1", bufs=8, space="PSUM") as ps1,
            ):
                tgate = ph1.tile([128, JC, BL], F32, tag="tgate")
                tgrep = ph1.tile([128, JC, TPC * BL], BF, tag="tgrep")

                # --- 1a: target-embedding gather + tgate (frees wiht after) --
                with tc.tile_pool(name="pwiht", bufs=1) as pwiht:
                    wiht_sb = pwiht.tile([128, K, JC, 128], BF)
                    nc.sync.dma_start(wiht_sb[:], wiht_d[:].rearrange(
                        "p (k jc m) -> p k jc m", k=K, jc=JC))
                    tidx_sb = cload(pwiht, tidx_d, [128, 8], I16)

                    txT = pwiht.tile([128, K, 128], BF)
                    nc.gpsimd.dma_gather(
                        txT[:], temb[:], tidx_sb[:, :], num_idxs=128,
                        num_idxs_reg=128, elem_size=D, transpose=True)
                    if "txT" in tap_d:
                        nc.sync.dma_start(
                            tap_d["txT"][:], txT[:].rearrange("p k c -> p (k c)"))

                    for jc in range(JC):
                        pt = ps1.tile([128, BL], F32, tag="ps")
                        for k in range(K):
                            nc.tensor.matmul(
                                pt[:], wiht_sb[:, k, jc, :], txT[:, k, :BL],
                                start=(k == 0), stop=(k == K - 1))
                        nc.vector.tensor_scalar_add(
                            tgate[:, jc, :], pt[:], blstm_sb[:, jc : jc + 1])
                    if "tgate" in tap_d:
                        nc.sync.dma_start(
                            tap_d["tgate"][:], tgate[:].rearrange("p a b -> p (a b)"))
                    for r in range(TPC):
                        nc.vector.tensor_copy(
                            tgrep[:, :, r * BL : (r + 1) * BL], tgate[:])

                # --- 1b: X gather + combine + xpre ---
                with (
                    tc.tile_pool(name="pwihx", bufs=1) as pwihx,
                    tc.tile_pool(name="pgat", bufs=1) as pgat,
                    tc.tile_pool(name="pxgs", bufs=3) as pxgs,
                ):
                    wihx_sb = pwihx.tile([128, K, JC, 128], BF)
                    nc.sync.dma_start(wihx_sb[:], wihx_d[:].rearrange(
                        "p (k jc m) -> p k jc m", k=K, jc=JC))
                    idxlo_sb = cload(pwihx, idxlo_d, [128, COLS // 16], I16)
                    idxhi_sb = cload(pwihx, idxhi_d, [128, COLS // 16], I16)
                    m0_sb = cload(pwihx, m0_d, [128, COLS], BF)
                    XT = pwihx.tile([128, K, COLS], BF)

                    emb_lo = emb[0:VSPLIT, :]
                    emb_hi = emb[VSPLIT:VOCAB, :]
                    for c in range(NCT):
                        g0 = pgat.tile([128, K, CW], BF, tag="g0")
                        g1 = pgat.tile([128, K, CW], BF, tag="g1")
                        iw = CW // 16
                        nc.gpsimd.dma_gather(
                            g0[:], emb_lo, idxlo_sb[:, c * iw : (c + 1) * iw],
                            num_idxs=CW, num_idxs_reg=CW, elem_size=D,
                            transpose=True)
                        nc.gpsimd.dma_gather(
                            g1[:], emb_hi, idxhi_sb[:, c * iw : (c + 1) * iw],
                            num_idxs=CW, num_idxs_reg=CW, elem_size=D,
                            transpose=True)
                        # X^T[:,k,cs] = g1 + (g0-g1)*m0
                        for k in range(K):
                            cs = slice(c * CW, (c + 1) * CW)
                            tmp = pgat.tile([128, CW], BF, tag="cmb")
                            nc.vector.tensor_sub(tmp[:], g0[:, k, :], g1[:, k, :])
                            nc.vector.tensor_mul(tmp[:], tmp[:], m0_sb[:, cs])
                            nc.vector.tensor_add(XT[:, k, cs], tmp[:], g1[:, k, :])
                    if "XT" in tap_d:
                        nc.sync.dma_start(
                            tap_d["XT"][:], XT[:].rearrange("p k c -> p (k c)"))

                    # xpre: xg^T[jc,(t,b)] = sum_k Wihx[k,jc]^T @ X^T[k] (+tg)
                    # split into column halves so the second half's gathers
                    # overlap the first half's matmuls
                    HC = max(1, NCT // 2)
                    for half in range(NCT // HC):
                        crange = range(half * HC, (half + 1) * HC)
                        for jc in range(JC):
                            pcs = [ps1.tile([128, CW], F32, tag="ps",
                                            name=f"pcs{c_}") for c_ in crange]
                            for k in range(K):
                                for i, c in enumerate(crange):
                                    nc.tensor.matmul(
                                        pcs[i][:], wihx_sb[:, k, jc, :],
                                        XT[:, k, c * CW : (c + 1) * CW],
                                        start=(k == 0), stop=(k == K - 1))
                            for i, c in enumerate(crange):
                                xs = pxgs.tile([128, CW], BF, tag="xgs")
                                nc.vector.tensor_add(
                                    xs[:].rearrange("p (t b) -> p t b", b=BL),
                                    pcs[i][:].rearrange("p (t b) -> p t b", b=BL),
                                    tgrep[:, jc, :].rearrange(
                                        "p (t b) -> p t b", b=BL))
                                nc.sync.dma_start(
                                    xg_dram[jc, :, c * CW : (c + 1) * CW], xs[:])

            if "xg" in tap_d:
                nc.sync.dma_start(tap_d["xg"][:], xg_dram[:])

            # ================= phase 2: recurrence =========================
            with (
                tc.tile_pool(name="pwhh", bufs=1) as pwhh,
                tc.tile_pool(name="pxgb", bufs=2) as pxgb,
                tc.tile_pool(name="pcell", bufs=2) as pcell,
                tc.tile_pool(name="ps_g", bufs=2, space="PSUM") as ps_g,
            ):
                whh_sb = pwhh.tile([128, K, JC, 128], F8)
                nc.sync.dma_start(whh_sb[:], whh_d[:].rearrange(
                    "p (k jc m) -> p k jc m", k=K, jc=JC))

                # channel-chunk groups: group g covers h/c chunks
                # cc in [g*CCW,(g+1)*CCW); gate column chunks jc = gate*K+cc.
                # Elementwise for group g runs while the PE computes group
                # g+1's matmuls; next step's k-chunk matmuls start as soon
                # as h chunk k lands.
                NG = NG_H
                CCW = CCW_H

                c_prevs = []
                for g in range(NG):
                    cz = pcell.tile([128, K // NG, BL], F32, tag=f"c{g}",
                                    name="cz")
                    nc.vector.memset(cz[:], 0.0)
                    c_prevs.append(cz)

                xgb = None
                for t in range(TT):
                    ct, tl = divmod(t, TPC)
                    if tl == 0:
                        xgb = pxgb.tile([128, JC, CW], BF, tag="xgb")
                        nc.sync.dma_start(
                            xgb[:],
                            xg_dram[:, :, ct * CW : (ct + 1) * CW].rearrange(
                                "jc p c -> p jc c"))
                    # [128, 4(gate), K(cc), BL]
                    xg_t = xgb[:, :, tl * BL : (tl + 1) * BL].rearrange(
                        "p (g cc) b -> p g cc b", g=4)

                    psgs = [None] * NG
                    if t > 0:
                        # NOTE: start=True clears has_written at PSUM *bank*
                        # granularity, so each jc's 6-matmul accumulation
                        # group must be issued consecutively.
                        for g in range(NG):
                            psg = ps_g.tile([128, 4, CCW, BL], F32,
                                            tag=f"psg{g}", name=f"psg{g}")
                            psgs[g] = psg
                            for gate in range(4):
                                for ci, cc in enumerate(
                                        range(g * CCW, (g + 1) * CCW)):
                                    jc = gate * K + cc
                                    for k in range(K):
                                        nc.tensor.matmul(
                                            psg[:, gate, ci, :],
                                            whh_sb[:, k, jc, :],
                                            Vhk(k)[:, :, t - 1],
                                            start=(k == 0), stop=(k == K - 1))

                    c_news = []
                    for g in range(NG):
                        sl = slice(g * CCW, (g + 1) * CCW)
                        gates = pcell.tile([128, 4, CCW, BL], F32,
                                           tag=f"gates{g}", name=f"gates{g}")
                        if t == 0:
                            nc.vector.tensor_copy(gates[:], xg_t[:, :, sl, :])
                        else:
                            nc.vector.tensor_add(gates[:], psgs[g][:],
                                                 xg_t[:, :, sl, :])
                        sigif = pcell.tile([128, 2, CCW, BL], F32,
                                           tag=f"sigif{g}", name="sigif")
                        nc.scalar.activation(sigif[:], gates[:, 0:2],
                                             AF.Sigmoid)
                        tang = pcell.tile([128, CCW, BL], F32, tag=f"tang{g}",
                                          name="tang")
                        nc.scalar.activation(tang[:], gates[:, 2], AF.Tanh)
                        sigo = pcell.tile([128, CCW, BL], F32, tag=f"sigo{g}",
                                          name="sigo")
                        nc.scalar.activation(sigo[:], gates[:, 3], AF.Sigmoid)
                        t1 = pcell.tile([128, CCW, BL], F32, tag=f"t1{g}",
                                        name="t1")
                        nc.vector.tensor_mul(t1[:], sigif[:, 1], c_prevs[g][:])
                        t2 = pcell.tile([128, CCW, BL], F32, tag=f"t2{g}",
                                        name="t2")
                        nc.vector.tensor_mul(t2[:], sigif[:, 0], tang[:])
                        c_new = pcell.tile([128, CCW, BL], F32, tag=f"c{g}",
                                           name="c_new")
                        nc.vector.tensor_add(c_new[:], t1[:], t2[:])
                        tanc = pcell.tile([128, CCW, BL], F32, tag=f"tanc{g}",
                                          name="tanc")
                        nc.scalar.activation(tanc[:], c_new[:], AF.Tanh)
                        nc.vector.tensor_mul(
                            Vhg[g][:, :, :, t], sigo[:], tanc[:])
                        c_news.append(c_new)
                    c_prevs = c_news

                if "HT" in tap_d:
                    tapv = tap_d["HT"][:].rearrange(
                        "p (k c) -> p k c", k=K)
                    for k in range(K):
                        nc.sync.dma_start(tapv[:, k, :], HTk(k))

            # ================= phase 3: attention + head ===================
            with (
                tc.tile_pool(name="pattw", bufs=1) as pattw,
                tc.tile_pool(name="pthw", bufs=1) as pthw,
                tc.tile_pool(name="prows", bufs=1) as prows,
                tc.tile_pool(name="pattn", bufs=1) as pattn,
                tc.tile_pool(name="ps3", bufs=4, space="PSUM") as ps3,
            ):
                wh_sb = pattw.tile([128, K, K, 128], BF, tag="wh")
                nc.sync.dma_start(wh_sb[:], wh_d[:].rearrange(
                    "p (k jc m) -> p k jc m", k=K, jc=K))
                wp_sb = pattw.tile([128, K, K, 128], BF, tag="wp")
                nc.sync.dma_start(wp_sb[:], wp_d[:].rearrange(
                    "p (k jc m) -> p k jc m", k=K, jc=K))
                wx_sb = pattw.tile([128, K, K, 128], BF, tag="wx")
                nc.sync.dma_start(wx_sb[:], wx_d[:].rearrange(
                    "p (k jc m) -> p k jc m", k=K, jc=K))

                # H rows per b (t on partitions) via DMA transpose,
                # split across both HWDGE queues; overlaps the HWh matmuls
                hrows = prows.tile([TT, BL, D], BF)
                identb = None
                if TT % 128 != 0:
                    identb = pattn.tile([128, 128], BF, tag="identb")
                    nc.vector.tensor_copy(identb[:], ident_sb[:])
                for b in range(BL):
                    for k in range(K):
                        if TT % 128 == 0:
                            nc.sync.dma_start(
                                hrows[:, b, k * 128 : (k + 1) * 128],
                                HTk(k)[:, b * TT : (b + 1) * TT],
                                transpose=True)
                        else:
                            ptp = ps3.tile([TT, 128], BF, tag="psb",
                                           name="ptp")
                            nc.tensor.transpose(
                                ptp[:], HTk(k)[:, b * TT : (b + 1) * TT],
                                identb[:, :])
                            nc.vector.tensor_copy(
                                hrows[:, b, k * 128 : (k + 1) * 128], ptp[:])

                # tanh(H Wh)^T  [128, K, COLS]
                thw = pthw.tile([128, K, COLS], BF)
                for jc in range(K):
                    for c in range(NCT):
                        ph = ps3.tile([128, CW], F32, tag="ps")
                        for k in range(K):
                            nc.tensor.matmul(
                                ph[:], wh_sb[:, k, jc, :],
                                HTk(k)[:, c * CW : (c + 1) * CW],
                                start=(k == 0), stop=(k == K - 1))
                        nc.scalar.activation(
                            thw[:, jc, c * CW : (c + 1) * CW], ph[:], AF.Tanh)

                # scores[(b,t)] = sum_{jc,p} thw[p,jc,col] * w[jc*128+p]
                scf = pattn.tile([1, COLS], F32, tag="scf")
                for c in range(NCT):
                    psc = ps3.tile([1, CW], F32, tag="ps")
                    for jc in range(K):
                        nc.tensor.matmul(
                            psc[:], wvec_sb[:, jc : jc + 1],
                            thw[:, jc, c * CW : (c + 1) * CW],
                            start=(jc == 0), stop=(jc == K - 1))
                    nc.scalar.copy(scf[:, c * CW : (c + 1) * CW], psc[:])

                # reshape [1,(b,t)] -> [b,t] via DRAM bounce, mask, softmax
                nc.sync.dma_start(sc_dram[:], scf[:])
                sc_bt = pattn.tile([BL, TT], F32, tag="scbt")
                nc.sync.dma_start(
                    sc_bt[:],
                    sc_dram[:].rearrange("o (b t) -> o b t", b=BL)[0])
                nc.vector.tensor_add(sc_bt[:], sc_bt[:], mbias_sb[:])
                if "scores" in tap_d:
                    nc.sync.dma_start(tap_d["scores"][:], sc_bt[:])
                mx = pattn.tile([BL, 1], F32, tag="mx")
                nc.vector.reduce_max(mx[:], sc_bt[:], axis=mybir.AxisListType.X)
                nc.vector.tensor_scalar_sub(sc_bt[:], sc_bt[:], mx[:])
                nc.scalar.activation(sc_bt[:], sc_bt[:], AF.Exp)
                sm = pattn.tile([BL, 1], F32, tag="sm")
                nc.vector.reduce_sum(sm[:], sc_bt[:], axis=mybir.AxisListType.X)
                nc.vector.reciprocal(sm[:], sm[:])
                nc.vector.tensor_scalar_mul(sc_bt[:], sc_bt[:], sm[:])
                if "alpha" in tap_d:
                    nc.sync.dma_start(tap_d["alpha"][:], sc_bt[:])

                # alpha^T via PE transpose
                pal = ps3.tile([TT, BL], F32, tag="ps")
                nc.tensor.transpose(pal[:], sc_bt[:], ident_sb[:BL, :BL])
                alphaT = pattn.tile([TT, BL], BF, tag="alphaT")
                nc.vector.tensor_copy(alphaT[:], pal[:])

                # block-diagonal [alpha | onehot] stationary: A [T, BL*64]
                A = pattn.tile([TT, BL * 64], BF, tag="A")
                nc.vector.memset(A[:], 0.0)
                nc.vector.tensor_copy(A[:, 0 : BL * 64 : 65], alphaT[:])
                nc.vector.tensor_copy(A[:, 32 : BL * 64 : 65], onehot_sb[:TT, :])

                # [r ; h_last] rows = sum_b A_b^T @ Hrows_b
                pr1 = ps3.tile([64, 512], F32, tag="ps")
                pr2 = ps3.tile([64, D - 512], F32, tag="ps")
                for b in range(BL):
                    Ab = A[:, b * 64 : (b + 1) * 64]
                    nc.tensor.matmul(pr1[:], Ab, hrows[:, b, 0:512],
                                     start=(b == 0), stop=(b == BL - 1))
                    nc.tensor.matmul(pr2[:], Ab, hrows[:, b, 512:D],
                                     start=(b == 0), stop=(b == BL - 1))
                rh = pattn.tile([64, D], F32, tag="rh")
                nc.vector.tensor_copy(rh[:, 0:512], pr1[:])
                nc.vector.tensor_copy(rh[:, 512:D], pr2[:])
                if "rh" in tap_d:
                    nc.sync.dma_start(tap_d["rh"][:], rh[:])

                # rh^T [128, K, 64] via PE transposes
                rhT = pattn.tile([128, K, 64], BF, tag="rhT")
                for k in range(K):
                    prt = ps3.tile([128, 64], F32, tag="ps")
                    nc.tensor.transpose(
                        prt[:], rh[:, k * 128 : (k + 1) * 128],
                        ident_sb[:64, :64])
                    nc.vector.tensor_copy(rhT[:, k, :], prt[:])

                # h_star^T = tanh(Wp^T r^T + Wx^T hlast^T)
                hstarT = pattn.tile([128, K, BL], BF, tag="hstarT")
                for jc in range(K):
                    phs = ps3.tile([128, BL], F32, tag="ps")
                    for k in range(K):
                        nc.tensor.matmul(phs[:], wp_sb[:, k, jc, :],
                                         rhT[:, k, 0:BL],
                                         start=(k == 0), stop=False)
                    for k in range(K):
                        nc.tensor.matmul(phs[:], wx_sb[:, k, jc, :],
                                         rhT[:, k, BL:64],
                                         start=False, stop=(k == K - 1))
                    nc.scalar.activation(hstarT[:, jc, :], phs[:], AF.Tanh)
                if "hstar" in tap_d:
                    nc.sync.dma_start(
                        tap_d["hstar"][:],
                        hstarT[:].rearrange("p k b -> p (k b)"))

                # logits^T [3, BL]
                pl = ps3.tile([NCLS, BL], F32, tag="ps")
                for k in range(K):
                    nc.tensor.matmul(pl[:], wlin_sb[:, k, :], hstarT[:, k, :],
                                     start=(k == 0), stop=(k == K - 1))
                logit = pattn.tile([NCLS, BL], F32, tag="logit")
                nc.vector.tensor_scalar_add(logit[:], pl[:], blin_sb[:])
                nc.sync.dma_start(out_d[:], logit[:])

    nc.finalize()
    return nc


# ======================= host-side wrapper =============================

_CACHE = {}


def _img_kjc(w, jc):
    # [768, jc*128] -> [128, K*jc*128] SBUF image, [p, k, jc, m]
    k = w.shape[0] // 128
    return np.ascontiguousarray(
        w.reshape(k, 128, jc, 128).transpose(1, 0, 2, 3).reshape(128, -1))


def prep_inputs(sent, target, lens, emb, temb, W_ih, W_hh, b_lstm, Wh, Wv, w,
                Wp, Wx, W_lin, b_lin, t_steps=T):
    TT = t_steps
    COLS = _cols(TT)
    b16 = lambda x: np.asarray(x, np.float32).astype(BF16NP)

    shared = {
        "emb": b16(emb),
        "temb": b16(temb),
        "whh": _img_kjc(np.asarray(W_hh, np.float32).astype(F8NP), JC),
        "wihx": _img_kjc(b16(W_ih[:D]), JC),
        "wiht": _img_kjc(b16(W_ih[D:]), JC),
        "wh": _img_kjc(b16(Wh), K),
        "wp": _img_kjc(b16(Wp), K),
        "wx": _img_kjc(b16(Wx), K),
        "wlin": np.ascontiguousarray(
            b16(W_lin).reshape(K, 128, NCLS).transpose(1, 0, 2).reshape(128, -1)),
        "wvec": np.ascontiguousarray(b16(w[:D]).reshape(K, 128).T),
        "blstm": np.ascontiguousarray(
            np.asarray(b_lstm, np.float32).reshape(JC, 128).T),
        "blin": np.asarray(b_lin, np.float32).reshape(NCLS, 1),
        "ident": np.eye(128, dtype=np.float32),
    }

    sent = np.asarray(sent)
    target = np.asarray(target)
    lens = np.asarray(lens)

    def wrap16(flat):
        # [n] -> [128, n//16]: wrapped in 16 partitions, replicated into all
        # 8 GpSimd-core stripes (each Q7 core reads its own 16-partition band)
        return np.ascontiguousarray(np.tile(flat.reshape(-1, 16).T, (8, 1)))

    in_maps = []
    for c in range(NCORES):
        sl = slice(c * BL, (c + 1) * BL)
        s = sent[sl, :TT]
        flat = s.T.reshape(-1).astype(np.int64)  # col = t*BL + b
        lo = np.where(flat < VSPLIT, flat, 0).astype(np.int16)
        hi = np.where(flat >= VSPLIT, flat - VSPLIT, 0).astype(np.int16)
        m0 = np.broadcast_to(
            (flat < VSPLIT).astype(BF16NP)[None, :], (128, COLS)).copy()
        tflat = np.zeros(128, np.int64)
        tflat[:BL] = target[sl]
        ln = np.clip(lens[sl].astype(np.int64), 1, TT)
        mbias = np.where(np.arange(TT)[None, :] < ln[:, None], 0.0, -1e9
                         ).astype(np.float32)
        onehot = (np.arange(128)[:, None] == (ln - 1)[None, :]).astype(BF16NP)
        m = dict(shared)
        m.update({
            "idxlo": wrap16(lo), "idxhi": wrap16(hi), "m0": m0,
            "tidx": wrap16(tflat.astype(np.int16)),
            "mbias": mbias, "onehot": onehot,
        })
        in_maps.append(m)
    return in_maps


def _run(inputs, t_steps=T, taps=(), trace=False):
    from concourse import bass_utils

    if trace:
        _install_profile_shim()
    key = (t_steps, tuple(sorted(taps)))
    if key not in _CACHE:
        _CACHE[key] = build(t_steps=t_steps, taps=taps)
    nc = _CACHE[key]
    in_maps = prep_inputs(t_steps=t_steps, **inputs)
    res = bass_utils.run_bass_kernel_spmd(
        nc, in_maps, core_ids=list(range(NCORES)), trace=trace)
    logits = np.zeros((B, NCLS), np.float32)
    for c in range(NCORES):
        logits[c * BL : (c + 1) * BL] = res.results[c]["out"].T
    return logits, res


def kernel(**inputs):
    logits, _ = _run(inputs)
    return logits


def _install_profile_shim():
    import contextlib, ctypes, types
    import antenv

    if "antenv.axon_hooks" in sys.modules:
        return
    so = "/opt/axon/libaxon_pjrt.so"
    try:
        lib = ctypes.CDLL(so)
        lib.axon_start_nrt_profile.argtypes = [
            ctypes.POINTER(ctypes.c_int64), ctypes.c_size_t]
        lib.axon_start_nrt_profile.restype = ctypes.c_int64
        lib.axon_stop_nrt_profile.argtypes = [ctypes.c_char_p]
        lib.axon_stop_nrt_profile.restype = ctypes.c_int64
    except OSError:
        return

    @contextlib.contextmanager
    def _hook(output_dir, device_ids):
        import jax
        jax.devices()
        if device_ids:
            ids = (ctypes.c_int64 * len(device_ids))(*device_ids)
            rc = lib.axon_start_nrt_profile(ids, len(device_ids))
        else:
            rc = lib.axon_start_nrt_profile(None, 0)
        if rc != 0:
            raise RuntimeError(f"axon_start_nrt_profile rc={rc}")
        try:
            yield
        finally:
            n = lib.axon_stop_nrt_profile(str(output_dir).encode())
            print(f"ntff profile: {n} file(s) -> {output_dir}", file=sys.stderr)

    mod = types.ModuleType("antenv.axon_hooks")
    mod.get_axon_ntff_profile_hook = lambda: _hook
    mod.set_axon_ntff_profile_hook = lambda h: None
    sys.modules["antenv.axon_hooks"] = mod
    antenv.axon_hooks = mod
